# revision 1
# baseline (speedup 1.0000x reference)
"""Trainium2 Bass kernel for windowed 3D cross-attention (CrossAttention3D).

Reference computation (see problem statement):
  (B=1, C=128, D=H=W=48) q/k/v maps -> channels-last -> partition into
  6x6x6 windows (8x8x8 = 512 windows, 216 tokens each) -> LayerNorm over C
  (q with gamma_q/beta_q, k and v with gamma_kv/beta_kv) -> 8-head
  attention within each window (dh=16) -> output projection -> window
  reverse -> (B, C, D, H, W).

Sharding: data-parallel over the window depth axis. Core m processes the
D-slab d in [6m, 6m+6) — 64 independent windows per core. Params are
replicated.

Per-core pipeline (all f32):
  - loop over 8 "eighths" (one window-row of 8 windows): DMA the
    [128, 6, 6, 48] q/k/v slabs to SBUF
  - per window: PE-transpose [C, tok] -> [tok, C] in 2 chunks of 108
    tokens; bn_stats/bn_aggr for LN stats; rsqrt via bit-trick Newton on
    DVE; normalize with per-partition scalars
  - scores^T = kn^T.T @ qn^T per head (K=16 matmuls, row-tiled via
    tile_position; odd heads use a DMA-shifted copy of the transposed
    tiles because PE partition bases must be 32-aligned)
  - softmax without max-subtraction: one Exp per (chunk, parity) over a
    multi-bank PSUM view; denominators come free from an appended
    ones-column in the v operand (M=17 matmuls, col-tiled)
  - rescale by 1/Z (DMA partition-broadcast + fast reciprocal), then the
    projection as two accumulating K=128 matmuls against host-prepadded
    W^T halves; bias folded into the output copy
  - output staged in SBUF and DMA'd back per eighth
"""
import sys

sys.path.insert(0, "/opt/trn_rl_repo")

from contextlib import ExitStack

import numpy as np

import concourse.bass as bass
import concourse.tile as tile
from concourse import bacc, mybir
from concourse.bass_utils import run_bass_kernel_spmd
from concourse.masks import make_identity

F32 = mybir.dt.float32
C = 128          # channels
NH = 8           # heads
DH = 16          # head dim
T = 216          # tokens per window (6*6*6)
TC = 108         # tokens per chunk (3 d-slices)
NCORES = 8

_BUILD_CACHE = {}
# bisection knobs (full kernel: 8, 8, 6)
N_EIGHTHS = 8
N_WINDOWS = 8
STAGE = 60


def _build_nc(trivial_q: bool, trivial_kv: bool, trivial_bias: bool):
    key = (trivial_q, trivial_kv, trivial_bias, N_EIGHTHS, N_WINDOWS, STAGE)
    if key in _BUILD_CACHE:
        return _BUILD_CACHE[key]

    nc = bacc.Bacc("TRN2", target_bir_lowering=False, debug=False,
                   num_devices=NCORES)
    qs = nc.dram_tensor("q_slab", [C, 6, 48, 48], F32, kind="ExternalInput")
    ks = nc.dram_tensor("k_slab", [C, 6, 48, 48], F32, kind="ExternalInput")
    vs = nc.dram_tensor("v_slab", [C, 6, 48, 48], F32, kind="ExternalInput")
    wt_ev = nc.dram_tensor("wt_ev", [C, C], F32, kind="ExternalInput")
    wt_od = nc.dram_tensor("wt_od", [C, C], F32, kind="ExternalInput")
    pb = nc.dram_tensor("pbias", [C, 1], F32, kind="ExternalInput")
    gq = bq = gkv = bkv = None
    if not trivial_q:
        gq = nc.dram_tensor("gq_b", [C, C], F32, kind="ExternalInput")
        bq = nc.dram_tensor("bq_b", [C, C], F32, kind="ExternalInput")
    if not trivial_kv:
        gkv = nc.dram_tensor("gkv_b", [C, C], F32, kind="ExternalInput")
        bkv = nc.dram_tensor("bkv_b", [C, C], F32, kind="ExternalInput")
    ys = nc.dram_tensor("y_slab", [C, 6, 48, 48], F32, kind="ExternalOutput")

    with tile.TileContext(nc) as tc, ExitStack() as ctx:
        consts = ctx.enter_context(tc.tile_pool(name="consts", bufs=1))
        inp = ctx.enter_context(tc.tile_pool(name="inp", bufs=2))
        outp = ctx.enter_context(tc.tile_pool(name="outp", bufs=2))
        sb_tc = ctx.enter_context(tc.tile_pool(name="sb_tc", bufs=4))
        sb_small = ctx.enter_context(tc.tile_pool(name="sb_small", bufs=4))
        sb_T = ctx.enter_context(tc.tile_pool(name="sb_T", bufs=2))
        sb_E = ctx.enter_context(tc.tile_pool(name="sb_E", bufs=3))
        sb_o = ctx.enter_context(tc.tile_pool(name="sb_o", bufs=2))
        pp_tp = ctx.enter_context(tc.tile_pool(name="pp_tp", bufs=1, space="PSUM"))
        pp_bt = ctx.enter_context(tc.tile_pool(name="pp_bt", bufs=1, space="PSUM"))
        pp_sc = ctx.enter_context(tc.tile_pool(name="pp_sc", bufs=2, space="PSUM"))
        pp_av = ctx.enter_context(tc.tile_pool(name="pp_av", bufs=1, space="PSUM"))
        pp_zb = ctx.enter_context(tc.tile_pool(name="pp_zb", bufs=1, space="PSUM"))

        ident = consts.tile([C, C], F32)
        make_identity(nc, ident[:])
        ones32 = consts.tile([1, 32], F32)
        nc.vector.memset(ones32[:], 1.0)
        t_wt_ev = consts.tile([C, C], F32)
        t_wt_od = consts.tile([C, C], F32)
        nc.sync.dma_start(t_wt_ev[:], wt_ev[:, :])
        nc.sync.dma_start(t_wt_od[:], wt_od[:, :])
        t_pb = None
        if not trivial_bias:
            t_pb = consts.tile([C, 1], F32)
            nc.sync.dma_start(t_pb[:], pb[:, :])
        t_gq = t_bq = t_gkv = t_bkv = None
        if not trivial_q:
            t_gq = consts.tile([C, C], F32)
            t_bq = consts.tile([C, C], F32)
            nc.sync.dma_start(t_gq[:], gq[:, :])
            nc.sync.dma_start(t_bq[:], bq[:, :])
        if not trivial_kv:
            t_gkv = consts.tile([C, C], F32)
            t_bkv = consts.tile([C, C], F32)
            nc.sync.dma_start(t_gkv[:], gkv[:, :])
            nc.sync.dma_start(t_bkv[:], bkv[:, :])

        def rsqrt_cols(var_view, out_tile, eps):
            """out = 1/sqrt(var+eps) elementwise; bit-trick seed + 2 Newton."""
            p, f = out_tile.shape[0], out_tile.free_size()
            ve = sb_small.tile([p, f], F32, tag="rs_ve")
            nc.vector.tensor_scalar_add(ve[:], var_view, float(eps))
            ti = sb_small.tile([p, f], mybir.dt.int32, tag="rs_ti")
            nc.vector.tensor_scalar(
                ti[:], ve[:].bitcast(mybir.dt.int32), 1, None,
                op0=mybir.AluOpType.logical_shift_right)
            tn = sb_small.tile([p, f], mybir.dt.int32, tag="rs_tn")
            nc.vector.tensor_scalar(
                tn[:], ti[:], 0, None, op0=mybir.AluOpType.bitwise_not)
            ts_ = sb_small.tile([p, f], mybir.dt.int32, tag="rs_ts")
            nc.vector.tensor_scalar(
                ts_[:], tn[:], 0x5f3759df + 1, None, op0=mybir.AluOpType.add)
            y_cur = ts_[:].bitcast(F32)
            t1 = sb_small.tile([p, f], F32, tag="rs_t1")
            for it in range(2):
                nc.vector.tensor_tensor(t1[:], y_cur, y_cur, op=mybir.AluOpType.mult)
                nc.vector.tensor_tensor(t1[:], t1[:], ve[:], op=mybir.AluOpType.mult)
                nc.vector.tensor_scalar(t1[:], t1[:], -0.5, 1.5,
                                        op0=mybir.AluOpType.mult,
                                        op1=mybir.AluOpType.add)
                dst = out_tile if it == 1 else sb_small.tile([p, f], F32, tag="rs_y")
                nc.vector.tensor_tensor(dst[:], t1[:], y_cur, op=mybir.AluOpType.mult)
                y_cur = dst[:]

        for ihw in range(N_EIGHTHS):
            t_q = inp.tile([C, 6, 6, 48], F32, tag="in_q")
            t_k = inp.tile([C, 6, 6, 48], F32, tag="in_k")
            t_v = inp.tile([C, 6, 6, 48], F32, tag="in_v")
            nc.sync.dma_start(t_q[:], qs[:, :, 6 * ihw:6 * ihw + 6, :])
            nc.sync.dma_start(t_k[:], ks[:, :, 6 * ihw:6 * ihw + 6, :])
            nc.sync.dma_start(t_v[:], vs[:, :, 6 * ihw:6 * ihw + 6, :])
            t_oe = outp.tile([C, 6, 6, 48], F32)
            if STAGE < 60 or N_WINDOWS < 8:
                nc.gpsimd.memset(t_oe[:], 0.0)

            for iww in range(N_WINDOWS):
                w0 = 6 * iww
                srcs = (t_q, t_k, t_v)
                # ---- LN phase (per 108-token chunk) ---------------------------
                qn_c, kn_c, vg_c = [], [], []
                for jc in range(2):
                    tp3 = pp_tp.tile([TC, 3, C], F32, tag="tp")
                    stats = sb_small.tile([TC, 6], F32, tag="stats")
                    for it in range(3):
                        view = srcs[it][:, 3 * jc:3 * jc + 3, :, w0:w0 + 6]
                        gat = sb_small.tile([C, TC], F32, tag="gat")
                        nc.gpsimd.tensor_copy(gat[:], view)
                        nc.tensor.transpose(tp3[:, it, :], gat[:], ident[:])
                    # single whole-tile PSUM read (same-bank PE-W/DVE-R safety),
                    # then everything downstream works from SBUF
                    xt = sb_tc.tile([TC, 3, C], F32, tag="xt")
                    nc.vector.tensor_copy(xt[:], tp3[:])
                    for it in range(3):
                        st6 = sb_small.tile([TC, 6], F32, tag="bn6")
                        nc.vector.bn_stats(st6[:], xt[:, it, :])
                        nc.vector.bn_aggr(stats[:, 2 * it:2 * it + 2], st6[:])
                    rstd = sb_small.tile([TC, 3], F32, tag="rstd")
                    var_view = stats[:].rearrange("p (a b) -> p a b", b=2)[:, :, 1]
                    rsqrt_cols(var_view, rstd, 1e-5)

                    t_qn = sb_tc.tile([TC, C], F32, tag="qn")
                    t_kn = sb_tc.tile([TC, C], F32, tag="kn")
                    t_vg = sb_tc.tile([TC, NH, 17], F32, tag="vg")
                    qn_c.append(t_qn)
                    kn_c.append(t_kn)
                    vg_c.append(t_vg)
                    norm_dst = (t_qn[:], t_kn[:], t_vg[:, :, 0:16])
                    gb = ((t_gq, t_bq), (t_gkv, t_bkv), (t_gkv, t_bkv))
                    triv = (trivial_q, trivial_kv, trivial_kv)
                    for it in range(3):
                        mean = stats[:, 2 * it:2 * it + 1]
                        rs = rstd[:, it:it + 1]
                        if triv[it]:
                            nc.vector.tensor_scalar(
                                norm_dst[it], xt[:, it, :], mean, rs,
                                op0=mybir.AluOpType.subtract,
                                op1=mybir.AluOpType.mult)
                        else:
                            tmp = sb_tc.tile([TC, C], F32, tag="norm_tmp")
                            nc.vector.tensor_scalar(
                                tmp[:], xt[:, it, :], mean, rs,
                                op0=mybir.AluOpType.subtract,
                                op1=mybir.AluOpType.mult)
                            g_t, b_t = gb[it]
                            nc.vector.tensor_tensor(
                                tmp[:], tmp[:], g_t[:TC, :],
                                op=mybir.AluOpType.mult)
                            nc.vector.tensor_tensor(
                                norm_dst[it], tmp[:], b_t[:TC, :],
                                op=mybir.AluOpType.add)
                    nc.gpsimd.memset(t_vg[:, :, 16:17], 1.0)

                if STAGE < 2:
                    continue
                # ---- transposed qn/kn -----------------------------------------
                # layout [C, qk, jc, TC]: q tokens contiguous across jc
                t_qkT = sb_T.tile([C, 2, 2, TC], F32, tag="qkT")
                for jc in range(2):
                    btp = pp_bt.tile([C, 2, TC], F32, tag="bt")
                    nc.tensor.transpose(btp[:, 0, :], qn_c[jc][:], ident[:TC, :TC])
                    nc.tensor.transpose(btp[:, 1, :], kn_c[jc][:], ident[:TC, :TC])
                    # single read of the whole PSUM tile
                    nc.vector.tensor_copy(t_qkT[:, :, jc, :], btp[:])
                qkT_h = []
                for h in range(NH):
                    th = sb_T.tile([16, 2, 2, TC], F32, tag=f"qkTh{h}")
                    nc.sync.dma_start(th[:], t_qkT[16 * h:16 * h + 16, :, :, :])
                    qkT_h.append(th)

                if STAGE < 25:
                    continue
                # ---- scores + softmax numerators ------------------------------
                e_c = []
                for jc in range(2):
                    t_E = sb_E.tile([TC, NH, T], F32, tag="E")
                    e_c.append(t_E)
                    for par in range(2):
                        sc = pp_sc.tile([TC, 4, 256], F32, tag="sc")
                        for g in range(4):
                            th = qkT_h[2 * g + par]
                            q_rhs = th[:, 0, :, :].rearrange("p a b -> p (a b)")
                            nc.tensor.matmul(
                                sc[:, g, 0:T],
                                th[:, 1, jc, :],
                                q_rhs,
                                start=True, stop=True)
                        if STAGE >= 30:
                            e_view = t_E[:].rearrange(
                                "p (a b) t -> p a b t", b=2)[:, :, par, :]
                            nc.scalar.activation(
                                e_view, sc[:, :, 0:T],
                                mybir.ActivationFunctionType.Exp, scale=0.25)
                        else:
                            nc.vector.tensor_copy(
                                t_E[:, 0:4, :], sc[:, :, 0:T])

                if STAGE < 40:
                    continue
                # ---- attn @ v (with ones column -> Z row) ---------------------
                t_av = pp_av.tile([C, 2, T], F32, tag="av")
                nc.vector.memset(t_av[:], 0.0)
                for h in range(NH):
                    g = h // 2
                    for jc in range(2):
                        nc.tensor.matmul(
                            t_av[32 * g:32 * g + 17, h % 2, :],
                            vg_c[jc][:, h, :],
                            e_c[jc][:, h, :],
                            start=(jc == 0), stop=(jc == 1),
                            tile_position=(0, 32 * g))

                if STAGE < 50:
                    continue
                # ---- 1/Z rescale + head gather --------------------------------
                t_o = sb_o.tile([C, 2, T], F32, tag="o")
                t_zrA = sb_o.tile([1, 4 * T], F32, tag="zrA")
                t_zrB = sb_o.tile([1, 4 * T], F32, tag="zrB")
                t_ziA = sb_o.tile([C, T], F32, tag="ziA")
                t_ziB = sb_o.tile([C, T], F32, tag="ziB")
                # single whole-tile PSUM read, ordered after all 16 matmuls
                nc.vector.tensor_copy(t_o[:], t_av[:])
                for xi, (zrow, zbi) in enumerate(
                        ((t_zrA, t_ziA), (t_zrB, t_ziB))):
                    # stage the 4 Z rows (partitions 16,48,80,112) on partition 0
                    for g in range(4):
                        nc.sync.dma_start(zrow[0:1, 216 * g:216 * g + 216],
                                          t_o[32 * g + 16:32 * g + 17, xi, :])
                    # broadcast each Z row over its 32-partition block via a
                    # K=1 outer product (col-tiled PE matmuls)
                    zb = pp_zb.tile([C, T], F32, tag="zb")
                    for g in range(4):
                        nc.tensor.matmul(
                            zb[32 * g:32 * g + 32, :], ones32[:],
                            zrow[0:1, 216 * g:216 * g + 216],
                            start=True, stop=True, tile_position=(0, 32 * g))
                    nc.vector.reciprocal_approx_fast(zbi[:], zb[:])
                    for g in range(4):
                        nc.vector.tensor_tensor(
                            t_o[32 * g:32 * g + 16, xi, :],
                            t_o[32 * g:32 * g + 16, xi, :],
                            zbi[32 * g:32 * g + 16, :],
                            op=mybir.AluOpType.mult)

                if STAGE < 60:
                    continue
                # ---- projection + output --------------------------------------
                y_ps = pp_bt.tile([C, 2, TC], F32, tag="bt")
                y_flat = y_ps[:].rearrange("p a b -> p (a b)")
                nc.tensor.matmul(y_flat, t_wt_ev[:], t_o[:, 0, :],
                                 start=True, stop=False)
                nc.tensor.matmul(y_flat, t_wt_od[:], t_o[:, 1, :],
                                 start=False, stop=True)
                out_view = t_oe[:, :, :, w0:w0 + 6]
                if trivial_bias:
                    nc.vector.tensor_copy(out_view, y_flat)
                else:
                    nc.vector.tensor_scalar_add(out_view, y_flat, t_pb[:, 0:1])

            nc.sync.dma_start(ys[:, :, 6 * ihw:6 * ihw + 6, :], t_oe[:])

    nc.compile()
    _BUILD_CACHE[key] = nc
    return nc


def _prepare(inputs):
    q_map = np.asarray(inputs["q_map"], np.float32)
    k_map = np.asarray(inputs["k_map"], np.float32)
    v_map = np.asarray(inputs["v_map"], np.float32)
    gamma_q = np.asarray(inputs["gamma_q"], np.float32)
    beta_q = np.asarray(inputs["beta_q"], np.float32)
    gamma_kv = np.asarray(inputs["gamma_kv"], np.float32)
    beta_kv = np.asarray(inputs["beta_kv"], np.float32)
    proj_w = np.asarray(inputs["proj_w"], np.float32)
    proj_b = np.asarray(inputs["proj_b"], np.float32)

    trivial_q = bool(np.all(gamma_q == 1.0) and np.all(beta_q == 0.0))
    trivial_kv = bool(np.all(gamma_kv == 1.0) and np.all(beta_kv == 0.0))
    trivial_bias = bool(np.all(proj_b == 0.0))

    wt = np.ascontiguousarray(proj_w.T)  # [c_in, c_out]
    wt_ev = np.zeros((C, C), np.float32)
    wt_od = np.zeros((C, C), np.float32)
    for g in range(4):
        wt_ev[32 * g:32 * g + 16] = wt[32 * g:32 * g + 16]
        wt_od[32 * g:32 * g + 16] = wt[32 * g + 16:32 * g + 32]

    in_maps = []
    for m in range(NCORES):
        im = {
            "q_slab": np.ascontiguousarray(q_map[0, :, 6 * m:6 * m + 6]),
            "k_slab": np.ascontiguousarray(k_map[0, :, 6 * m:6 * m + 6]),
            "v_slab": np.ascontiguousarray(v_map[0, :, 6 * m:6 * m + 6]),
            "wt_ev": wt_ev,
            "wt_od": wt_od,
            "pbias": np.ascontiguousarray(proj_b.reshape(C, 1)),
        }
        if not trivial_q:
            im["gq_b"] = np.ascontiguousarray(
                np.broadcast_to(gamma_q, (C, C)).astype(np.float32))
            im["bq_b"] = np.ascontiguousarray(
                np.broadcast_to(beta_q, (C, C)).astype(np.float32))
        if not trivial_kv:
            im["gkv_b"] = np.ascontiguousarray(
                np.broadcast_to(gamma_kv, (C, C)).astype(np.float32))
            im["bkv_b"] = np.ascontiguousarray(
                np.broadcast_to(beta_kv, (C, C)).astype(np.float32))
        in_maps.append(im)
    return (trivial_q, trivial_kv, trivial_bias), in_maps


def _run(inputs, trace=False, **trace_kwargs):
    flags, in_maps = _prepare(inputs)
    nc = _build_nc(*flags)
    res = run_bass_kernel_spmd(nc, in_maps, list(range(NCORES)),
                               trace=trace, **trace_kwargs)
    slabs = [res.results[m]["y_slab"] for m in range(NCORES)]
    out = np.concatenate(slabs, axis=1).reshape(1, C, 48, 48, 48)
    return out.astype(np.float32), res


def kernel(**inputs):
    out, _ = _run(inputs, trace=False)
    return out


def kernel_traced(**inputs):
    return _run(inputs, trace=True)



# revision 20
# speedup vs baseline: 1.7031x; 1.7031x over previous
"""Trainium2 Bass kernel for windowed 3D cross-attention (CrossAttention3D).

Reference computation:
  (B=1, C=128, D=H=W=48) q/k/v maps -> channels-last -> partition into
  6x6x6 windows (512 windows, 216 tokens each) -> LayerNorm over C ->
  8-head attention within each window (dh=16) -> output projection ->
  window reverse -> (B, C, D, H, W).

Sharding: data-parallel over the window depth axis. Core m processes the
D-slab d in [6m, 6m+6) -- 64 independent windows per core.

v2 design (see commit history for the all-fp32 baseline):
  - PE matmul cost is out_free_size x cycles_per_row and cycles_per_row
    is 4 for fp32 but 1 for fp16 -- all attention matmuls use fp16
    operands (PSUM accumulates fp32).
  - LN stats come from tiny K=128/N=1 PE matmuls in channel-major
    layout (lhsT = x chunk, rhs = ones column) -- no per-window
    transposes of q/k. Normalization applies broadcast stat rows
    (K=1 outer products into PSUM).
  - q^/k^ are DMA-restaged into head-major [16, 8, 216] tiles so every
    score matmul reads partition-base-0 operands (PE operands off base
    0/32/64 are rejected, and mixing tile row-positions within a PSUM
    bank faults at runtime).
  - v^ is PE-transposed to token-major (needed as the attn@v stationary
    operand anyway) and normalized with per-partition scalars; a
    constant ones column yields the softmax denominator Z as row 32g of
    the attn@v PSUM block.
  - exp on the Activation engine straight out of score PSUM; 1/Z rows
    are DMA-gathered to partition 0, broadcast with K=1 outer products,
    and applied in a single tensor_tensor.
  - gamma_kv/beta_kv fold into the projection weights/bias on the host
    (softmax rows sum to 1). gamma_q/beta_q (if nontrivial) are one
    per-partition tensor_scalar on q^.
"""
import sys

sys.path.insert(0, "/opt/trn_rl_repo")

from contextlib import ExitStack

import numpy as np

import concourse.bass as bass
import concourse.tile as tile
from concourse import bacc, mybir
from concourse.bass_utils import run_bass_kernel_spmd
from concourse.masks import make_identity

F32 = mybir.dt.float32
F32R = mybir.dt.float32r
F16 = mybir.dt.float16
I32 = mybir.dt.int32
C = 128          # channels
NH = 8           # heads
DH = 16          # head dim
T = 216          # tokens per window (6*6*6)
TC = 108         # tokens per chunk (3 d-slices)
NCORES = 8
EPS = 1e-5

_BUILD_CACHE = {}


def _build_nc(trivial_q: bool, trivial_bias: bool, DEBUG=False):
    key = (trivial_q, trivial_bias, DEBUG)
    if key in _BUILD_CACHE:
        return _BUILD_CACHE[key]

    nc = bacc.Bacc("TRN2", target_bir_lowering=False, debug=False,
                   num_devices=NCORES)
    qs = nc.dram_tensor("q_slab", [C, 8, 8, T], F32, kind="ExternalInput")
    ks = nc.dram_tensor("k_slab", [C, 8, 8, T], F32, kind="ExternalInput")
    vs = nc.dram_tensor("v_slab", [C, 8, 8, T], F32, kind="ExternalInput")
    wt0 = nc.dram_tensor("wt0", [C, C], F16, kind="ExternalInput")
    wt1 = nc.dram_tensor("wt1", [C, C], F16, kind="ExternalInput")
    pb = nc.dram_tensor("pbias", [C, 1], F32, kind="ExternalInput")
    gq = bq = None
    if not trivial_q:
        gq = nc.dram_tensor("gq", [C, 1], F32, kind="ExternalInput")
        bq = nc.dram_tensor("bq", [C, 1], F32, kind="ExternalInput")
    ys = nc.dram_tensor("y_slab", [C, 8, 8, T], F32, kind="ExternalOutput")
    dbg = {}
    if DEBUG:
        dbg["st"] = nc.dram_tensor("dbg_st", [TC, 2, 8, 6], F32, kind="ExternalOutput")
        dbg["rstd"] = nc.dram_tensor("dbg_rstd", [TC, 2, 8, 3], F32, kind="ExternalOutput")
        dbg["D"] = nc.dram_tensor("dbg_D", [TC, 2, 8, 4], F16, kind="ExternalOutput")
        dbg["qh"] = nc.dram_tensor("dbg_qh", [C, T], F16, kind="ExternalOutput")
        dbg["kh"] = nc.dram_tensor("dbg_kh", [C, T], F16, kind="ExternalOutput")
        dbg["qhH"] = nc.dram_tensor("dbg_qhH", [DH, NH, T], F16, kind="ExternalOutput")
        dbg["s4"] = nc.dram_tensor("dbg_s4", [1, 4, 2, TC], F16, kind="ExternalOutput")
        dbg["spb"] = nc.dram_tensor("dbg_spb", [4, 2, TC], F16, kind="ExternalOutput")
        dbg["tv"] = nc.dram_tensor("dbg_tv", [TC, 2, 2, 4, 32], F16, kind="ExternalOutput")
        dbg["E"] = nc.dram_tensor("dbg_E", [TC, 2, 2, 4, T], F16, kind="ExternalOutput")
        dbg["avn"] = nc.dram_tensor("dbg_avn", [C, 2, T], F16, kind="ExternalOutput")
        dbg["R"] = nc.dram_tensor("dbg_R", [C, 2, T], F32, kind="ExternalOutput")

    AF = mybir.ActivationFunctionType
    OP = mybir.AluOpType

    with tile.TileContext(nc) as tc, ExitStack() as ctx:
        consts = ctx.enter_context(tc.tile_pool(name="consts", bufs=1))
        inp = ctx.enter_context(tc.tile_pool(name="inp", bufs=2))
        outp = ctx.enter_context(tc.tile_pool(name="outp", bufs=2))
        sqp = ctx.enter_context(tc.tile_pool(name="sqp", bufs=2))
        qkp = ctx.enter_context(tc.tile_pool(name="qkp", bufs=2))
        qkh = ctx.enter_context(tc.tile_pool(name="qkh", bufs=3))
        tmpp = ctx.enter_context(tc.tile_pool(name="tmpp", bufs=2))
        sS = ctx.enter_context(tc.tile_pool(name="sS", bufs=2))
        ep = ctx.enter_context(tc.tile_pool(name="ep", bufs=2))
        avn_p = ctx.enter_context(tc.tile_pool(name="avn_p", bufs=2))
        rp = ctx.enter_context(tc.tile_pool(name="rp", bufs=2))
        # PSUM (8 banks x 2KB): scr 1 + bcn/bp 2 + big(sc,y) 2x2 + av 1
        p_scr = ctx.enter_context(tc.tile_pool(name="p_scr", bufs=1, space="PSUM"))
        p_bcn = ctx.enter_context(tc.tile_pool(name="p_bcn", bufs=1, space="PSUM"))
        p_big = ctx.enter_context(tc.tile_pool(name="p_big", bufs=2, space="PSUM"))
        p_av = ctx.enter_context(tc.tile_pool(name="p_av", bufs=1, space="PSUM"))

        ident = consts.tile([C, C], F32)
        make_identity(nc, ident[:])
        ident16 = consts.tile([C, C], F16)
        nc.vector.tensor_copy(ident16[:], ident[:])
        ones_col = consts.tile([C, 1], F32)
        nc.vector.memset(ones_col[:], 1.0)
        onesr16 = consts.tile([1, C], F16)
        nc.vector.memset(onesr16[:], 1.0)
        onesr32 = consts.tile([1, 32], F32)
        nc.vector.memset(onesr32[:], 1.0)
        t_wt = []
        for hh, w_dram in enumerate((wt0, wt1)):
            t_w = consts.tile([C, C], F16, tag=f"wt{hh}")
            nc.sync.dma_start(t_w[:], w_dram[:, :])
            t_wt.append(t_w)
        t_pb = None
        if not trivial_bias:
            t_pb = consts.tile([C, 1], F32)
            nc.sync.dma_start(t_pb[:], pb[:, :])
        t_gq = t_bq = None
        if not trivial_q:
            t_gq = consts.tile([C, 1], F32)
            t_bq = consts.tile([C, 1], F32)
            nc.sync.dma_start(t_gq[:], gq[:, :])
            nc.sync.dma_start(t_bq[:], bq[:, :])
        # Derived LN stats (fp16), one persistent tile, written per eighth.
        # Cols: (rq, rk, wq, wk); transposed per window so the 4 stat rows
        # land on partitions 0..3 (a partition-strided DMA gather silently
        # reads the wrong partitions, so keep everything contiguous).
        t_D = consts.tile([TC, 2, 8, 4], F16)
        # f32 per-token v stats (tensor_scalar scalars must be f32):
        # col 0 = rv, col 1 = mv
        t_Dv = consts.tile([TC, 2, 8, 2], F32)
        # v^ stationary tiles: [chunk, hh, g, 32]; col 0 = ones (Z row),
        # cols 1..16 = channels of head 4*hh+g, rest zero.
        t_vh = []
        for w in range(8):
            tv = consts.tile([TC, 2, 2, 4, 32], F16, tag=f"tv{w}", name=f"tv{w}")
            nc.vector.memset(tv[:], 0.0)
            nc.vector.memset(tv[:, :, :, :, 0:1], 1.0)
            t_vh.append(tv)

        def rsqrt_cols(var_view, out_view, scale):
            """out = scale/sqrt(var); bit-trick seed + 2 Newton (no ACT
            table, keeps the activation table pinned on Exp)."""
            p = TC
            ve = sS.tile([p, 2, 8, 3], F32, tag="rs_ve")
            nc.vector.tensor_copy(ve[:], var_view)
            ti = sS.tile([p, 2, 8, 3], I32, tag="rs_ti")
            nc.vector.tensor_scalar(
                ti[:], ve[:].bitcast(I32), 1, None,
                op0=OP.logical_shift_right)
            nc.vector.tensor_scalar(
                ti[:], ti[:], 0, None, op0=OP.bitwise_not)
            nc.vector.tensor_scalar(
                ti[:], ti[:], 0x5f3759df + 1, None, op0=OP.add)
            y_cur = ti[:].bitcast(F32)
            t1 = sS.tile([p, 2, 8, 3], F32, tag="rs_t1")
            for it in range(2):
                nc.vector.tensor_tensor(t1[:], y_cur, y_cur, op=OP.mult)
                nc.vector.tensor_tensor(t1[:], t1[:], ve[:], op=OP.mult)
                nc.vector.tensor_scalar(t1[:], t1[:], -0.5, 1.5,
                                        op0=OP.mult, op1=OP.add)
                if it == 0:
                    yn = sS.tile([p, 2, 8, 3], F32, tag="rs_yn")
                    nc.vector.tensor_tensor(yn[:], t1[:], y_cur, op=OP.mult)
                    y_cur = yn[:]
                else:
                    # fold the final scale into the last Newton multiply
                    nc.vector.tensor_tensor(t1[:], t1[:], y_cur, op=OP.mult)
                    nc.vector.tensor_scalar(out_view, t1[:], scale, None,
                                            op0=OP.mult)

        for ihw in range(8):
            t_q = inp.tile([C, 8, T], F32, tag="in_q")
            t_k = inp.tile([C, 8, T], F32, tag="in_k")
            t_v = inp.tile([C, 8, T], F32, tag="in_v")
            nc.sync.dma_start(t_q[:], qs[:, ihw, :, :])
            nc.sync.dma_start(t_k[:], ks[:, ihw, :, :])
            nc.sync.dma_start(t_v[:], vs[:, ihw, :, :])
            t_oe = outp.tile([C, 8, T], F32)

            # ---- stats: 12 tiny matmuls per window ------------------------
            # st cols: (q_s1, q_s2, k_s1, k_s2, v_s1, v_s2)
            st = p_scr.tile([TC, 2, 8, 6], F32, tag="scr")
            for iww in range(8):
                for it, srcT in enumerate((t_q, t_k, t_v)):
                    win = srcT[:, iww, :]
                    sq = sqp.tile([C, T], F32, tag=f"sq{it}")
                    nc.gpsimd.tensor_tensor(sq[:], win, win, op=OP.mult)
                    for jc in range(2):
                        nc.tensor.matmul(st[:, jc, iww, 2 * it:2 * it + 1],
                                         srcT[:, iww, TC * jc:TC * jc + TC],
                                         ones_col[:],
                                         start=True, stop=True)
                        nc.tensor.matmul(st[:, jc, iww, 2 * it + 1:2 * it + 2],
                                         sq[:, TC * jc:TC * jc + TC],
                                         ones_col[:],
                                         start=True, stop=True)

            # ---- derived stats (batched over the eighth) ------------------
            stS = sS.tile([TC, 2, 8, 6], F32, tag="stS")
            nc.vector.tensor_copy(stS[:], st[:])
            if DEBUG and ihw == 0:
                nc.sync.dma_start(dbg["st"][:, :, :, :], stS[:])
            rr = stS[:].rearrange("p a w (b c) -> p a w b c", c=2)
            s1v, s2v = rr[:, :, :, :, 0], rr[:, :, :, :, 1]
            t1 = sS.tile([TC, 2, 8, 3], F32, tag="t1")
            nc.vector.tensor_tensor(t1[:], s1v, s1v, op=OP.mult)
            u0 = sS.tile([TC, 2, 8, 3], F32, tag="u0")
            nc.vector.tensor_scalar(u0[:], s2v, 128.0, 128.0 * 128.0 * EPS,
                                    op0=OP.mult, op1=OP.add)
            nc.vector.tensor_tensor(u0[:], u0[:], t1[:], op=OP.subtract)
            # rstd = 128/sqrt(u0): qk pair -> D cols {0,1}, v -> Dv col 0
            rstd = sS.tile([TC, 2, 8, 3], F32, tag="rstd")
            rsqrt_cols(u0[:], rstd[:], 128.0)
            nc.vector.tensor_copy(t_D[:, :, :, 0:2], rstd[:, :, :, 0:2])
            nc.vector.tensor_copy(t_Dv[:, :, :, 0:1], rstd[:, :, :, 2:3])
            # means scaled by 1/128: (mq, mk) -> tmp, mv -> D col 2
            m3 = sS.tile([TC, 2, 8, 3], F32, tag="m3")
            nc.vector.tensor_scalar(m3[:], s1v, 1.0 / 128.0, None, op0=OP.mult)
            nc.vector.tensor_copy(t_Dv[:, :, :, 1:2], m3[:, :, :, 2:3])
            # w = mean * rstd -> D cols {2, 3}
            nc.vector.tensor_tensor(t_D[:, :, :, 2:4], m3[:, :, :, 0:2],
                                    rstd[:, :, :, 0:2], op=OP.mult)
            if DEBUG and ihw == 0:
                nc.sync.dma_start(dbg["rstd"][:, :, :, :], rstd[:])
                nc.sync.dma_start(dbg["D"][:, :, :, :], t_D[:])

            # ---- pipelined window loop: NORM(w) + ATTN(w-1) ---------------
            qhH_t, khH_t = [None] * 8, [None] * 8
            for step in range(9):
                if step < 8:
                    iww = step
                    # stat row transpose: [TC, 4] -> [4, 2, TC] (fp16)
                    sp = p_scr.tile([4, 2, TC], F16, tag="scr")
                    for jc in range(2):
                        nc.tensor.transpose(sp[:, jc, :], t_D[:, jc, iww, :],
                                            ident16[:TC, :TC])
                    # stage to SBUF (DMA cannot read PSUM), then fold the
                    # 4 stat rows (rq, rk, wq, wk) onto partition 0
                    spb = sS.tile([4, 2, TC], F16, tag="spb")
                    nc.vector.tensor_copy(spb[:], sp[:])
                    s4 = sS.tile([1, 4, 2, TC], F16, tag="s4")
                    nc.sync.dma_start(s4[0:1, :, :, :], spb[0:4, :, :])
                    # broadcast rows: bcn slices (0=rq, 1=wq, 2=rk, 3=wk)
                    bcn = p_big.tile([C, 4, 256], F32, tag="big")
                    for s, dst in ((0, 0), (2, 1), (1, 2), (3, 3)):
                        nc.tensor.matmul(bcn[:, dst, 0:T], onesr16[:],
                                         s4[0:1, s, :, :], start=True, stop=True)
                    # q^ = q * rq_b - wq_b (fp16), DVE
                    q_win = t_q[:, iww, :]
                    k_win = t_k[:, iww, :]
                    tmq = tmpp.tile([C, T], F32, tag="tmq")
                    nc.vector.tensor_tensor(tmq[:], q_win, bcn[:, 0, 0:T],
                                            op=OP.mult)
                    qh = qkp.tile([C, T], F16, tag="qh")
                    nc.vector.tensor_tensor(qh[:], tmq[:], bcn[:, 1, 0:T],
                                            op=OP.subtract)
                    if not trivial_q:
                        nc.vector.tensor_scalar(qh[:], qh[:], t_gq[:, 0:1],
                                                t_bq[:, 0:1],
                                                op0=OP.mult, op1=OP.add)
                    # k^ (gpsimd cannot read PSUM -> DVE)
                    tmk = tmpp.tile([C, T], F32, tag="tmk")
                    nc.vector.tensor_tensor(tmk[:], k_win, bcn[:, 2, 0:T],
                                            op=OP.mult)
                    kh = qkp.tile([C, T], F16, tag="kh")
                    nc.vector.tensor_tensor(kh[:], tmk[:], bcn[:, 3, 0:T],
                                            op=OP.subtract)
                    # restage head-major: [16, 8, 216], all heads at base 0
                    qhH = qkh.tile([DH, NH, T], F16, tag="qhH")
                    khH = qkh.tile([DH, NH, T], F16, tag="khH")
                    for h in range(NH):
                        nc.sync.dma_start(qhH[:, h, :], qh[DH * h:DH * h + DH, :])
                        nc.sync.dma_start(khH[:, h, :], kh[DH * h:DH * h + DH, :])
                    qhH_t[iww], khH_t[iww] = qhH, khH
                    if DEBUG and ihw == 0 and iww == 0:
                        nc.sync.dma_start(dbg["qh"][:, :], qh[:])
                        nc.sync.dma_start(dbg["kh"][:, :], kh[:])
                        nc.sync.dma_start(dbg["qhH"][:, :, :], qhH[:])
                        nc.sync.dma_start(dbg["s4"][:, :, :, :], s4[:])
                        nc.sync.dma_start(dbg["spb"][:, :, :], spb[:])
                    # v^: transpose to token-major, normalize per-partition
                    vt = p_scr.tile([TC, 2, C], F32, tag="scr")
                    for jc in range(2):
                        v_chunk = t_v[:, iww, TC * jc:TC * jc + TC]
                        nc.tensor.transpose(vt[:, jc, :], v_chunk, ident[:])
                    tv = t_vh[iww]
                    for jc in range(2):
                        src = vt[:, jc, :].rearrange("p (r g d) -> p r g d",
                                                     r=2, g=4)
                        nc.vector.tensor_scalar(
                            tv[:, jc, :, :, 1:17], src,
                            t_Dv[:, jc, iww, 1:2], t_Dv[:, jc, iww, 0:1],
                            op0=OP.subtract, op1=OP.mult)

                if step >= 1:
                    iww = step - 1
                    if DEBUG and ihw == 0 and iww == 0:
                        nc.sync.dma_start(dbg["tv"][:, :, :, :, :], t_vh[0][:])
                    qhH, khH = qhH_t[iww], khH_t[iww]
                    tv = t_vh[iww]
                    t_E = ep.tile([TC, 2, 2, 4, T], F16, tag="E")
                    for jc in range(2):
                        for hh in range(2):
                            sc = p_big.tile([C, 4, 256], F32, tag="big")
                            for g in range(4):
                                h = 4 * hh + g
                                nc.tensor.matmul(
                                    sc[0:TC, g, 0:T],
                                    khH[:, h, TC * jc:TC * jc + TC],
                                    qhH[:, h, :], start=True, stop=True)
                            nc.scalar.activation(t_E[:, jc, hh, :, :],
                                                 sc[0:TC, :, 0:T],
                                                 AF.Exp, scale=0.25)
                    if DEBUG and ihw == 0 and iww == 0:
                        nc.sync.dma_start(dbg["E"][:, :, :, :, :], t_E[:])
                    # attn @ v; ones col makes row 32g the Z row
                    av = p_av.tile([C, 2, T], F32, tag="av")
                    for hh in range(2):
                        for g in range(4):
                            for jc in range(2):
                                nc.tensor.matmul(
                                    av[32 * g:32 * g + 32, hh, :],
                                    tv[:, jc, hh, g, :],
                                    t_E[:, jc, hh, g, :],
                                    start=(jc == 0), stop=(jc == 1),
                                    tile_position=(0, 32 * g))
                    # 1/Z -> gather Z-recip rows to partition 0 -> broadcast
                    t_R = rp.tile([C, 2, T], F32, tag="R")
                    nc.vector.reciprocal_approx_fast(
                        t_R[:].rearrange("p a b -> p (a b)"),
                        av[:].rearrange("p a b -> p (a b)"))
                    # fp16 copy on the (otherwise idle) gpsimd engine; the
                    # broadcast outer products need fp16 operands for
                    # 1 cycle/row (f32r needs producer-side rounding)
                    t_Rh = rp.tile([C, 2, T], F16, tag="Rh")
                    nc.gpsimd.tensor_copy(t_Rh[:], t_R[:])
                    r4 = rp.tile([1, 4, 2, T], F16, tag="r4")
                    for g in range(4):
                        nc.sync.dma_start(r4[0:1, g, :, :],
                                          t_Rh[32 * g:32 * g + 1, :, :])
                    bp = p_bcn.tile([C, 4, 256], F32, tag="bcn")
                    bpf = bp[:].rearrange("p a b -> p (a b)")
                    for g in range(4):
                        nc.tensor.matmul(
                            bpf[32 * g:32 * g + 32, 0:2 * T],
                            onesr16[0:1, 0:32],
                            r4[0:1, g, :, :].rearrange("p a b -> p (a b)"),
                            start=True, stop=True, tile_position=(0, 32 * g))
                    # TT cannot read two PSUM operands and divide is not a
                    # valid TT op: stage av to SBUF (ACT), then multiply by
                    # the 1/Z broadcast (DVE)
                    avS = avn_p.tile([C, 2, T], F16, tag="avS")
                    nc.scalar.copy(avS[:], av[:])
                    avn = avn_p.tile([C, 2, T], F16, tag="avn")
                    bpv = bpf[:, 0:2 * T].rearrange("p (a b) -> p a b", b=T)
                    nc.vector.tensor_tensor(avn[:], avS[:], bpv, op=OP.mult)
                    if DEBUG and ihw == 0 and iww == 0:
                        nc.sync.dma_start(dbg["avn"][:, :, :], avn[:])
                        nc.sync.dma_start(dbg["R"][:, :, :], t_R[:])
                    # projection
                    y = p_big.tile([C, 4, 256], F32, tag="big")
                    nc.tensor.matmul(y[:, 0, 0:T], t_wt[0][:], avn[:, 0, :],
                                     start=True, stop=False)
                    nc.tensor.matmul(y[:, 0, 0:T], t_wt[1][:], avn[:, 1, :],
                                     start=False, stop=True)
                    out_view = t_oe[:, iww, :]
                    if trivial_bias:
                        nc.scalar.copy(out_view, y[:, 0, 0:T])
                    else:
                        nc.vector.tensor_scalar(out_view, y[:, 0, 0:T],
                                                t_pb[:, 0:1], None, op0=OP.add)

            nc.sync.dma_start(ys[:, ihw, :, :], t_oe[:])

    nc.compile()
    _BUILD_CACHE[key] = nc
    return nc


def _prepare(inputs):
    q_map = np.asarray(inputs["q_map"], np.float32)
    k_map = np.asarray(inputs["k_map"], np.float32)
    v_map = np.asarray(inputs["v_map"], np.float32)
    gamma_q = np.asarray(inputs["gamma_q"], np.float32)
    beta_q = np.asarray(inputs["beta_q"], np.float32)
    gamma_kv = np.asarray(inputs["gamma_kv"], np.float32)
    beta_kv = np.asarray(inputs["beta_kv"], np.float32)
    proj_w = np.asarray(inputs["proj_w"], np.float32)
    proj_b = np.asarray(inputs["proj_b"], np.float32)

    trivial_q = bool(np.all(gamma_q == 1.0) and np.all(beta_q == 0.0))
    trivial_kv = bool(np.all(gamma_kv == 1.0) and np.all(beta_kv == 0.0))
    if not trivial_kv:
        raise NotImplementedError(
            "nontrivial gamma_kv/beta_kv on k not implemented")

    # gamma_kv folds into the projection weight columns; beta_kv adds
    # proj_w @ beta_kv to every output (softmax rows sum to 1).
    wt_v = proj_w.T * gamma_kv[:, None]   # [c_in, c_out]
    bias = proj_b + proj_w @ beta_kv
    trivial_bias = bool(np.all(bias == 0.0))

    # packed av-row layout: row 32g+1+d (hh slot) = channel 16*(4hh+g)+d
    wt0 = np.zeros((C, C), np.float32)
    wt1 = np.zeros((C, C), np.float32)
    for g in range(4):
        for d in range(DH):
            wt0[32 * g + 1 + d] = wt_v[DH * (4 * 0 + g) + d]
            wt1[32 * g + 1 + d] = wt_v[DH * (4 * 1 + g) + d]

    def to_windows(x, m):
        # [C, 6, 48, 48] -> [C, 8hw, 8ww, 216] with token order (d, h, w)
        s = x[0, :, 6 * m:6 * m + 6]
        s = s.reshape(C, 6, 8, 6, 8, 6)
        s = np.transpose(s, (0, 2, 4, 1, 3, 5))
        return np.ascontiguousarray(s.reshape(C, 8, 8, T))

    in_maps = []
    for m in range(NCORES):
        im = {
            "q_slab": to_windows(q_map, m),
            "k_slab": to_windows(k_map, m),
            "v_slab": to_windows(v_map, m),
            "wt0": wt0.astype(np.float16),
            "wt1": wt1.astype(np.float16),
            "pbias": np.ascontiguousarray(bias.reshape(C, 1)),
        }
        if not trivial_q:
            im["gq"] = np.ascontiguousarray(gamma_q.reshape(C, 1))
            im["bq"] = np.ascontiguousarray(beta_q.reshape(C, 1))
        in_maps.append(im)
    return (trivial_q, trivial_bias), in_maps


def _run(inputs, trace=False, **trace_kwargs):
    flags, in_maps = _prepare(inputs)
    nc = _build_nc(*flags)
    res = run_bass_kernel_spmd(nc, in_maps, list(range(NCORES)),
                               trace=trace, **trace_kwargs)
    slabs = []
    for m in range(NCORES):
        s = res.results[m]["y_slab"].reshape(C, 8, 8, 6, 6, 6)
        s = np.transpose(s, (0, 3, 1, 4, 2, 5)).reshape(C, 6, 48, 48)
        slabs.append(s)
    out = np.concatenate(slabs, axis=1).reshape(1, C, 48, 48, 48)
    return out.astype(np.float32), res


def kernel(**inputs):
    out, _ = _run(inputs, trace=False)
    return out


def kernel_traced(**inputs):
    return _run(inputs, trace=True)


# revision 21
# speedup vs baseline: 2.2812x; 1.3394x over previous
"""Trainium2 Bass kernel for windowed 3D cross-attention (CrossAttention3D).

Reference computation:
  (B=1, C=128, D=H=W=48) q/k/v maps -> channels-last -> partition into
  6x6x6 windows (512 windows, 216 tokens each) -> LayerNorm over C ->
  8-head attention within each window (dh=16) -> output projection ->
  window reverse -> (B, C, D, H, W).

Sharding: data-parallel over the window depth axis. Core m processes the
D-slab d in [6m, 6m+6) -- 64 independent windows per core.

v2 design (see commit history for the all-fp32 baseline):
  - PE matmul cost is out_free_size x cycles_per_row and cycles_per_row
    is 4 for fp32 but 1 for fp16 -- all attention matmuls use fp16
    operands (PSUM accumulates fp32).
  - LN stats come from tiny K=128/N=1 PE matmuls in channel-major
    layout (lhsT = x chunk, rhs = ones column) -- no per-window
    transposes of q/k. Normalization applies broadcast stat rows
    (K=1 outer products into PSUM).
  - q^/k^ are DMA-restaged into head-major [16, 8, 216] tiles so every
    score matmul reads partition-base-0 operands (PE operands off base
    0/32/64 are rejected, and mixing tile row-positions within a PSUM
    bank faults at runtime).
  - v^ is PE-transposed to token-major (needed as the attn@v stationary
    operand anyway) and normalized with per-partition scalars; a
    constant ones column yields the softmax denominator Z as row 32g of
    the attn@v PSUM block.
  - exp on the Activation engine straight out of score PSUM; 1/Z rows
    are DMA-gathered to partition 0, broadcast with K=1 outer products,
    and applied in a single tensor_tensor.
  - gamma_kv/beta_kv fold into the projection weights/bias on the host
    (softmax rows sum to 1). gamma_q/beta_q (if nontrivial) are one
    per-partition tensor_scalar on q^.
"""
import sys

sys.path.insert(0, "/opt/trn_rl_repo")

from contextlib import ExitStack

import numpy as np

import concourse.bass as bass
import concourse.tile as tile
from concourse import bacc, mybir
from concourse.bass_utils import run_bass_kernel_spmd
from concourse.masks import make_identity

F32 = mybir.dt.float32
F32R = mybir.dt.float32r
F16 = mybir.dt.float16
I32 = mybir.dt.int32
C = 128          # channels
NH = 8           # heads
DH = 16          # head dim
T = 216          # tokens per window (6*6*6)
TC = 108         # tokens per chunk (3 d-slices)
NCORES = 8
EPS = 1e-5

_BUILD_CACHE = {}


def _build_nc(trivial_q: bool, trivial_bias: bool, DEBUG=False):
    key = (trivial_q, trivial_bias, DEBUG)
    if key in _BUILD_CACHE:
        return _BUILD_CACHE[key]

    nc = bacc.Bacc("TRN2", target_bir_lowering=False, debug=False,
                   num_devices=NCORES)
    qs = nc.dram_tensor("q_slab", [C, 8, 8, T], F32, kind="ExternalInput")
    ks = nc.dram_tensor("k_slab", [C, 8, 8, T], F32, kind="ExternalInput")
    vs = nc.dram_tensor("v_slab", [C, 8, 8, T], F32, kind="ExternalInput")
    wt0 = nc.dram_tensor("wt0", [C, C], F16, kind="ExternalInput")
    wt1 = nc.dram_tensor("wt1", [C, C], F16, kind="ExternalInput")
    pb = nc.dram_tensor("pbias", [C, 1], F32, kind="ExternalInput")
    gq = bq = None
    if not trivial_q:
        gq = nc.dram_tensor("gq", [C, 1], F32, kind="ExternalInput")
        bq = nc.dram_tensor("bq", [C, 1], F32, kind="ExternalInput")
    ys = nc.dram_tensor("y_slab", [C, 8, 8, T], F32, kind="ExternalOutput")
    dbg = {}
    if DEBUG:
        dbg["st"] = nc.dram_tensor("dbg_st", [TC, 2, 8, 6], F32, kind="ExternalOutput")
        dbg["rstd"] = nc.dram_tensor("dbg_rstd", [TC, 2, 8, 3], F32, kind="ExternalOutput")
        dbg["D"] = nc.dram_tensor("dbg_D", [TC, 2, 8, 4], F16, kind="ExternalOutput")
        dbg["qh"] = nc.dram_tensor("dbg_qh", [C, T], F16, kind="ExternalOutput")
        dbg["kh"] = nc.dram_tensor("dbg_kh", [C, T], F16, kind="ExternalOutput")
        dbg["qhH"] = nc.dram_tensor("dbg_qhH", [DH, NH, T], F16, kind="ExternalOutput")
        dbg["s4"] = nc.dram_tensor("dbg_s4", [1, 4, 2, TC], F16, kind="ExternalOutput")
        dbg["spb"] = nc.dram_tensor("dbg_spb", [4, 2, TC], F16, kind="ExternalOutput")
        dbg["tv"] = nc.dram_tensor("dbg_tv", [TC, 2, 2, 4, 32], F16, kind="ExternalOutput")
        dbg["E"] = nc.dram_tensor("dbg_E", [TC, 2, 2, 4, T], F16, kind="ExternalOutput")
        dbg["avn"] = nc.dram_tensor("dbg_avn", [C, 2, T], F16, kind="ExternalOutput")
        dbg["R"] = nc.dram_tensor("dbg_R", [C, 2, T], F32, kind="ExternalOutput")

    AF = mybir.ActivationFunctionType
    OP = mybir.AluOpType

    with tile.TileContext(nc) as tc, ExitStack() as ctx:
        consts = ctx.enter_context(tc.tile_pool(name="consts", bufs=1))
        inp = ctx.enter_context(tc.tile_pool(name="inp", bufs=2))
        outp = ctx.enter_context(tc.tile_pool(name="outp", bufs=2))
        sqp = ctx.enter_context(tc.tile_pool(name="sqp", bufs=2))
        qkp = ctx.enter_context(tc.tile_pool(name="qkp", bufs=2))
        qkh = ctx.enter_context(tc.tile_pool(name="qkh", bufs=3))
        tmpp = ctx.enter_context(tc.tile_pool(name="tmpp", bufs=2))
        sS = ctx.enter_context(tc.tile_pool(name="sS", bufs=2))
        ep = ctx.enter_context(tc.tile_pool(name="ep", bufs=2))
        avn_p = ctx.enter_context(tc.tile_pool(name="avn_p", bufs=2))
        rp = ctx.enter_context(tc.tile_pool(name="rp", bufs=2))
        # PSUM (8 banks x 2KB): scr 1 + bcn/bp 2 + big(sc,y) 2x2 + av 1
        p_scr = ctx.enter_context(tc.tile_pool(name="p_scr", bufs=1, space="PSUM"))
        p_bcn = ctx.enter_context(tc.tile_pool(name="p_bcn", bufs=1, space="PSUM"))
        p_big = ctx.enter_context(tc.tile_pool(name="p_big", bufs=2, space="PSUM"))
        p_av = ctx.enter_context(tc.tile_pool(name="p_av", bufs=1, space="PSUM"))

        ident = consts.tile([C, C], F32)
        make_identity(nc, ident[:])
        ident16 = consts.tile([C, C], F16)
        nc.vector.tensor_copy(ident16[:], ident[:])
        ones_col = consts.tile([C, 1], F32)
        nc.vector.memset(ones_col[:], 1.0)
        onesr16 = consts.tile([1, C], F16)
        nc.vector.memset(onesr16[:], 1.0)
        onesr32 = consts.tile([1, 32], F32)
        nc.vector.memset(onesr32[:], 1.0)
        t_wt = []
        for hh, w_dram in enumerate((wt0, wt1)):
            t_w = consts.tile([C, C], F16, tag=f"wt{hh}")
            nc.sync.dma_start(t_w[:], w_dram[:, :])
            t_wt.append(t_w)
        t_pb = None
        if not trivial_bias:
            t_pb = consts.tile([C, 1], F32)
            nc.sync.dma_start(t_pb[:], pb[:, :])
        t_gq = t_bq = None
        if not trivial_q:
            t_gq = consts.tile([C, 1], F32)
            t_bq = consts.tile([C, 1], F32)
            nc.sync.dma_start(t_gq[:], gq[:, :])
            nc.sync.dma_start(t_bq[:], bq[:, :])
        # Derived LN stats (fp16), one persistent tile, written per eighth.
        # Cols: (rq, rk, wq, wk); transposed per window so the 4 stat rows
        # land on partitions 0..3 (a partition-strided DMA gather silently
        # reads the wrong partitions, so keep everything contiguous).
        t_D = consts.tile([TC, 2, 8, 4], F16)
        # f32 per-token v stats (tensor_scalar scalars must be f32):
        # col 0 = rv, col 1 = mv
        t_Dv = consts.tile([TC, 2, 8, 2], F32)
        # v^ stationary tiles: [chunk, hh, g, 32]; col 0 = ones (Z row),
        # cols 1..16 = channels of head 4*hh+g, rest zero.
        t_vh = []
        for w in range(8):
            tv = consts.tile([TC, 2, 2, 4, 32], F16, tag=f"tv{w}", name=f"tv{w}")
            nc.vector.memset(tv[:], 0.0)
            nc.vector.memset(tv[:, :, :, :, 0:1], 1.0)
            t_vh.append(tv)

        def rsqrt_cols(var_view, out_view, scale):
            """out = scale/sqrt(var); bit-trick seed + 2 Newton (no ACT
            table, keeps the activation table pinned on Exp)."""
            p = TC
            ve = sS.tile([p, 2, 8, 3], F32, tag="rs_ve")
            nc.vector.tensor_copy(ve[:], var_view)
            ti = sS.tile([p, 2, 8, 3], I32, tag="rs_ti")
            nc.vector.tensor_scalar(
                ti[:], ve[:].bitcast(I32), 1, None,
                op0=OP.logical_shift_right)
            nc.vector.tensor_scalar(
                ti[:], ti[:], 0, None, op0=OP.bitwise_not)
            nc.vector.tensor_scalar(
                ti[:], ti[:], 0x5f3759df + 1, None, op0=OP.add)
            y_cur = ti[:].bitcast(F32)
            t1 = sS.tile([p, 2, 8, 3], F32, tag="rs_t1")
            for it in range(2):
                nc.vector.tensor_tensor(t1[:], y_cur, y_cur, op=OP.mult)
                nc.vector.tensor_tensor(t1[:], t1[:], ve[:], op=OP.mult)
                nc.vector.tensor_scalar(t1[:], t1[:], -0.5, 1.5,
                                        op0=OP.mult, op1=OP.add)
                if it == 0:
                    yn = sS.tile([p, 2, 8, 3], F32, tag="rs_yn")
                    nc.vector.tensor_tensor(yn[:], t1[:], y_cur, op=OP.mult)
                    y_cur = yn[:]
                else:
                    # fold the final scale into the last Newton multiply
                    nc.vector.tensor_tensor(t1[:], t1[:], y_cur, op=OP.mult)
                    nc.vector.tensor_scalar(out_view, t1[:], scale, None,
                                            op0=OP.mult)

        for ihw in range(8):
            t_q = inp.tile([C, 8, T], F32, tag="in_q")
            t_k = inp.tile([C, 8, T], F32, tag="in_k")
            t_v = inp.tile([C, 8, T], F32, tag="in_v")
            nc.sync.dma_start(t_q[:], qs[:, ihw, :, :])
            nc.sync.dma_start(t_k[:], ks[:, ihw, :, :])
            nc.sync.dma_start(t_v[:], vs[:, ihw, :, :])
            t_oe = outp.tile([C, 8, T], F32)

            # ---- stats: 12 tiny matmuls per window ------------------------
            # st cols: (q_s1, q_s2, k_s1, k_s2, v_s1, v_s2)
            st = p_scr.tile([TC, 2, 8, 6], F32, tag="scr")
            for iww in range(8):
                for it, srcT in enumerate((t_q, t_k, t_v)):
                    win = srcT[:, iww, :]
                    sq = sqp.tile([C, T], F32, tag=f"sq{it}")
                    nc.vector.tensor_tensor(sq[:], win, win, op=OP.mult)
                    for jc in range(2):
                        nc.tensor.matmul(st[:, jc, iww, 2 * it:2 * it + 1],
                                         srcT[:, iww, TC * jc:TC * jc + TC],
                                         ones_col[:],
                                         start=True, stop=True)
                        nc.tensor.matmul(st[:, jc, iww, 2 * it + 1:2 * it + 2],
                                         sq[:, TC * jc:TC * jc + TC],
                                         ones_col[:],
                                         start=True, stop=True)

            # ---- derived stats (batched over the eighth) ------------------
            stS = sS.tile([TC, 2, 8, 6], F32, tag="stS")
            nc.vector.tensor_copy(stS[:], st[:])
            if DEBUG and ihw == 0:
                nc.sync.dma_start(dbg["st"][:, :, :, :], stS[:])
            rr = stS[:].rearrange("p a w (b c) -> p a w b c", c=2)
            s1v, s2v = rr[:, :, :, :, 0], rr[:, :, :, :, 1]
            t1 = sS.tile([TC, 2, 8, 3], F32, tag="t1")
            nc.vector.tensor_tensor(t1[:], s1v, s1v, op=OP.mult)
            u0 = sS.tile([TC, 2, 8, 3], F32, tag="u0")
            nc.vector.tensor_scalar(u0[:], s2v, 128.0, 128.0 * 128.0 * EPS,
                                    op0=OP.mult, op1=OP.add)
            nc.vector.tensor_tensor(u0[:], u0[:], t1[:], op=OP.subtract)
            # rstd = 128/sqrt(u0): qk pair -> D cols {0,1}, v -> Dv col 0
            rstd = sS.tile([TC, 2, 8, 3], F32, tag="rstd")
            rsqrt_cols(u0[:], rstd[:], 128.0)
            nc.vector.tensor_copy(t_D[:, :, :, 0:2], rstd[:, :, :, 0:2])
            nc.vector.tensor_copy(t_Dv[:, :, :, 0:1], rstd[:, :, :, 2:3])
            # means scaled by 1/128: (mq, mk) -> tmp, mv -> D col 2
            m3 = sS.tile([TC, 2, 8, 3], F32, tag="m3")
            nc.vector.tensor_scalar(m3[:], s1v, 1.0 / 128.0, None, op0=OP.mult)
            nc.vector.tensor_copy(t_Dv[:, :, :, 1:2], m3[:, :, :, 2:3])
            # w = mean * rstd -> D cols {2, 3}
            nc.vector.tensor_tensor(t_D[:, :, :, 2:4], m3[:, :, :, 0:2],
                                    rstd[:, :, :, 0:2], op=OP.mult)
            if DEBUG and ihw == 0:
                nc.sync.dma_start(dbg["rstd"][:, :, :, :], rstd[:])
                nc.sync.dma_start(dbg["D"][:, :, :, :], t_D[:])

            # ---- pipelined window loop: NORM(w) + ATTN(w-1) ---------------
            qhH_t = [None] * 8
            for step in range(9):
                if step < 8:
                    iww = step
                    # stat row transpose: [TC, 4] -> [4, 2, TC] (fp16)
                    sp = p_scr.tile([4, 2, TC], F16, tag="scr")
                    for jc in range(2):
                        nc.tensor.transpose(sp[:, jc, :], t_D[:, jc, iww, :],
                                            ident16[:TC, :TC])
                    # stage to SBUF (DMA cannot read PSUM), then fold the
                    # 4 stat rows (rq, rk, wq, wk) onto partition 0
                    spb = sS.tile([4, 2, TC], F16, tag="spb")
                    nc.vector.tensor_copy(spb[:], sp[:])
                    s4 = sS.tile([1, 4, 2, TC], F16, tag="s4")
                    nc.sync.dma_start(s4[0:1, :, :, :], spb[0:4, :, :])
                    # broadcast rows: bcn slices (0=rq, 1=wq, 2=rk, 3=wk)
                    bcn = p_big.tile([C, 4, 256], F32, tag="big")
                    for s, dst in ((0, 0), (2, 1), (1, 2), (3, 3)):
                        nc.tensor.matmul(bcn[:, dst, 0:T], onesr16[:],
                                         s4[0:1, s, :, :], start=True, stop=True)
                    # q^ = q * rq_b - wq_b (fp16), DVE
                    q_win = t_q[:, iww, :]
                    k_win = t_k[:, iww, :]
                    tmq = tmpp.tile([C, T], F32, tag="tmq")
                    nc.vector.tensor_tensor(tmq[:], q_win, bcn[:, 0, 0:T],
                                            op=OP.mult)
                    qkn = qkp.tile([C, 2, T], F16, tag="qkn")
                    nc.vector.tensor_tensor(qkn[:, 0, :], tmq[:], bcn[:, 1, 0:T],
                                            op=OP.subtract)
                    if not trivial_q:
                        nc.vector.tensor_scalar(qkn[:, 0, :], qkn[:, 0, :],
                                                t_gq[:, 0:1], t_bq[:, 0:1],
                                                op0=OP.mult, op1=OP.add)
                    # k^ (gpsimd cannot read PSUM -> DVE)
                    tmk = tmpp.tile([C, T], F32, tag="tmk")
                    nc.vector.tensor_tensor(tmk[:], k_win, bcn[:, 2, 0:T],
                                            op=OP.mult)
                    nc.vector.tensor_tensor(qkn[:, 1, :], tmk[:], bcn[:, 3, 0:T],
                                            op=OP.subtract)
                    # restage head-major (q and k together): [16, 8, 2, 216]
                    qkH = qkh.tile([DH, NH, 2, T], F16, tag="qkH")
                    for h in range(NH):
                        nc.sync.dma_start(qkH[:, h, :, :],
                                          qkn[DH * h:DH * h + DH, :, :])
                    qhH_t[iww] = qkH
                    if DEBUG and ihw == 0 and iww == 0:
                        nc.sync.dma_start(dbg["qh"][:, :], qkn[:, 0, :])
                        nc.sync.dma_start(dbg["kh"][:, :], qkn[:, 1, :])
                        nc.sync.dma_start(dbg["qhH"][:, :, :], qkH[:, :, 0, :])
                        nc.sync.dma_start(dbg["s4"][:, :, :, :], s4[:])
                        nc.sync.dma_start(dbg["spb"][:, :, :], spb[:])
                    # v^: transpose to token-major, normalize per-partition
                    vt = p_scr.tile([TC, 2, C], F32, tag="scr")
                    for jc in range(2):
                        v_chunk = t_v[:, iww, TC * jc:TC * jc + TC]
                        nc.tensor.transpose(vt[:, jc, :], v_chunk, ident[:])
                    tv = t_vh[iww]
                    for jc in range(2):
                        src = vt[:, jc, :].rearrange("p (r g d) -> p r g d",
                                                     r=2, g=4)
                        nc.vector.tensor_scalar(
                            tv[:, jc, :, :, 1:17], src,
                            t_Dv[:, jc, iww, 1:2], t_Dv[:, jc, iww, 0:1],
                            op0=OP.subtract, op1=OP.mult)

                if step >= 1:
                    iww = step - 1
                    if DEBUG and ihw == 0 and iww == 0:
                        nc.sync.dma_start(dbg["tv"][:, :, :, :, :], t_vh[0][:])
                    qkH = qhH_t[iww]
                    tv = t_vh[iww]
                    t_E = ep.tile([TC, 2, 2, 4, T], F16, tag="E")
                    for jc in range(2):
                        for hh in range(2):
                            sc = p_big.tile([C, 4, 256], F32, tag="big")
                            for g in range(4):
                                h = 4 * hh + g
                                nc.tensor.matmul(
                                    sc[0:TC, g, 0:T],
                                    qkH[:, h, 1, TC * jc:TC * jc + TC],
                                    qkH[:, h, 0, :], start=True, stop=True)
                            nc.scalar.activation(t_E[:, jc, hh, :, :],
                                                 sc[0:TC, :, 0:T],
                                                 AF.Exp, scale=0.25)
                    if DEBUG and ihw == 0 and iww == 0:
                        nc.sync.dma_start(dbg["E"][:, :, :, :, :], t_E[:])
                    # attn @ v; ones col makes row 32g the Z row
                    av = p_av.tile([C, 2, T], F32, tag="av")
                    for hh in range(2):
                        for g in range(4):
                            for jc in range(2):
                                nc.tensor.matmul(
                                    av[32 * g:32 * g + 32, hh, :],
                                    tv[:, jc, hh, g, :],
                                    t_E[:, jc, hh, g, :],
                                    start=(jc == 0), stop=(jc == 1),
                                    tile_position=(0, 32 * g))
                    # 1/Z -> gather Z-recip rows to partition 0 -> broadcast
                    t_R = rp.tile([C, 2, T], F32, tag="R")
                    nc.vector.reciprocal_approx_fast(
                        t_R[:].rearrange("p a b -> p (a b)"),
                        av[:].rearrange("p a b -> p (a b)"))
                    # fp16 copy on the (otherwise idle) gpsimd engine; the
                    # broadcast outer products need fp16 operands for
                    # 1 cycle/row (f32r needs producer-side rounding)
                    t_Rh = rp.tile([C, 2, T], F16, tag="Rh")
                    nc.scalar.copy(t_Rh[:], t_R[:])
                    r4 = rp.tile([1, 4, 2, T], F16, tag="r4")
                    for g in range(4):
                        nc.sync.dma_start(r4[0:1, g, :, :],
                                          t_Rh[32 * g:32 * g + 1, :, :])
                    bp = p_bcn.tile([C, 4, 256], F32, tag="bcn")
                    bpf = bp[:].rearrange("p a b -> p (a b)")
                    for g in range(4):
                        nc.tensor.matmul(
                            bpf[32 * g:32 * g + 32, 0:2 * T],
                            onesr16[0:1, 0:32],
                            r4[0:1, g, :, :].rearrange("p a b -> p (a b)"),
                            start=True, stop=True, tile_position=(0, 32 * g))
                    # TT cannot read two PSUM operands and divide is not a
                    # valid TT op: stage av to SBUF (ACT), then multiply by
                    # the 1/Z broadcast (DVE)
                    avS = avn_p.tile([C, 2, T], F16, tag="avS")
                    nc.scalar.copy(avS[:], av[:])
                    avn = avn_p.tile([C, 2, T], F16, tag="avn")
                    bpv = bpf[:, 0:2 * T].rearrange("p (a b) -> p a b", b=T)
                    nc.vector.tensor_tensor(avn[:], avS[:], bpv, op=OP.mult)
                    if DEBUG and ihw == 0 and iww == 0:
                        nc.sync.dma_start(dbg["avn"][:, :, :], avn[:])
                        nc.sync.dma_start(dbg["R"][:, :, :], t_R[:])
                    # projection
                    y = p_big.tile([C, 4, 256], F32, tag="big")
                    nc.tensor.matmul(y[:, 0, 0:T], t_wt[0][:], avn[:, 0, :],
                                     start=True, stop=False)
                    nc.tensor.matmul(y[:, 0, 0:T], t_wt[1][:], avn[:, 1, :],
                                     start=False, stop=True)
                    out_view = t_oe[:, iww, :]
                    if trivial_bias:
                        nc.scalar.copy(out_view, y[:, 0, 0:T])
                    else:
                        nc.vector.tensor_scalar(out_view, y[:, 0, 0:T],
                                                t_pb[:, 0:1], None, op0=OP.add)

            nc.sync.dma_start(ys[:, ihw, :, :], t_oe[:])

    nc.compile()
    _BUILD_CACHE[key] = nc
    return nc


def _prepare(inputs):
    q_map = np.asarray(inputs["q_map"], np.float32)
    k_map = np.asarray(inputs["k_map"], np.float32)
    v_map = np.asarray(inputs["v_map"], np.float32)
    gamma_q = np.asarray(inputs["gamma_q"], np.float32)
    beta_q = np.asarray(inputs["beta_q"], np.float32)
    gamma_kv = np.asarray(inputs["gamma_kv"], np.float32)
    beta_kv = np.asarray(inputs["beta_kv"], np.float32)
    proj_w = np.asarray(inputs["proj_w"], np.float32)
    proj_b = np.asarray(inputs["proj_b"], np.float32)

    trivial_q = bool(np.all(gamma_q == 1.0) and np.all(beta_q == 0.0))
    trivial_kv = bool(np.all(gamma_kv == 1.0) and np.all(beta_kv == 0.0))
    if not trivial_kv:
        raise NotImplementedError(
            "nontrivial gamma_kv/beta_kv on k not implemented")

    # gamma_kv folds into the projection weight columns; beta_kv adds
    # proj_w @ beta_kv to every output (softmax rows sum to 1).
    wt_v = proj_w.T * gamma_kv[:, None]   # [c_in, c_out]
    bias = proj_b + proj_w @ beta_kv
    trivial_bias = bool(np.all(bias == 0.0))

    # packed av-row layout: row 32g+1+d (hh slot) = channel 16*(4hh+g)+d
    wt0 = np.zeros((C, C), np.float32)
    wt1 = np.zeros((C, C), np.float32)
    for g in range(4):
        for d in range(DH):
            wt0[32 * g + 1 + d] = wt_v[DH * (4 * 0 + g) + d]
            wt1[32 * g + 1 + d] = wt_v[DH * (4 * 1 + g) + d]

    def to_windows(x, m):
        # [C, 6, 48, 48] -> [C, 8hw, 8ww, 216] with token order (d, h, w)
        s = x[0, :, 6 * m:6 * m + 6]
        s = s.reshape(C, 6, 8, 6, 8, 6)
        s = np.transpose(s, (0, 2, 4, 1, 3, 5))
        return np.ascontiguousarray(s.reshape(C, 8, 8, T))

    in_maps = []
    for m in range(NCORES):
        im = {
            "q_slab": to_windows(q_map, m),
            "k_slab": to_windows(k_map, m),
            "v_slab": to_windows(v_map, m),
            "wt0": wt0.astype(np.float16),
            "wt1": wt1.astype(np.float16),
            "pbias": np.ascontiguousarray(bias.reshape(C, 1)),
        }
        if not trivial_q:
            im["gq"] = np.ascontiguousarray(gamma_q.reshape(C, 1))
            im["bq"] = np.ascontiguousarray(beta_q.reshape(C, 1))
        in_maps.append(im)
    return (trivial_q, trivial_bias), in_maps


def _run(inputs, trace=False, **trace_kwargs):
    flags, in_maps = _prepare(inputs)
    nc = _build_nc(*flags)
    res = run_bass_kernel_spmd(nc, in_maps, list(range(NCORES)),
                               trace=trace, **trace_kwargs)
    slabs = []
    for m in range(NCORES):
        s = res.results[m]["y_slab"].reshape(C, 8, 8, 6, 6, 6)
        s = np.transpose(s, (0, 3, 1, 4, 2, 5)).reshape(C, 6, 48, 48)
        slabs.append(s)
    out = np.concatenate(slabs, axis=1).reshape(1, C, 48, 48, 48)
    return out.astype(np.float32), res


def kernel(**inputs):
    out, _ = _run(inputs, trace=False)
    return out


def kernel_traced(**inputs):
    return _run(inputs, trace=True)


# revision 23
# speedup vs baseline: 2.5188x; 1.1041x over previous
"""Trainium2 Bass kernel for windowed 3D cross-attention (CrossAttention3D).

Reference computation:
  (B=1, C=128, D=H=W=48) q/k/v maps -> channels-last -> partition into
  6x6x6 windows (512 windows, 216 tokens each) -> LayerNorm over C ->
  8-head attention within each window (dh=16) -> output projection ->
  window reverse -> (B, C, D, H, W).

Sharding: data-parallel over the window depth axis. Core m processes the
D-slab d in [6m, 6m+6) -- 64 independent windows per core.

v2 design (see commit history for the all-fp32 baseline):
  - PE matmul cost is out_free_size x cycles_per_row and cycles_per_row
    is 4 for fp32 but 1 for fp16 -- all attention matmuls use fp16
    operands (PSUM accumulates fp32).
  - LN stats come from tiny K=128/N=1 PE matmuls in channel-major
    layout (lhsT = x chunk, rhs = ones column) -- no per-window
    transposes of q/k. Normalization applies broadcast stat rows
    (K=1 outer products into PSUM).
  - q^/k^ are DMA-restaged into head-major [16, 8, 216] tiles so every
    score matmul reads partition-base-0 operands (PE operands off base
    0/32/64 are rejected, and mixing tile row-positions within a PSUM
    bank faults at runtime).
  - v^ is PE-transposed to token-major (needed as the attn@v stationary
    operand anyway) and normalized with per-partition scalars; a
    constant ones column yields the softmax denominator Z as row 32g of
    the attn@v PSUM block.
  - exp on the Activation engine straight out of score PSUM; 1/Z rows
    are DMA-gathered to partition 0, broadcast with K=1 outer products,
    and applied in a single tensor_tensor.
  - gamma_kv/beta_kv fold into the projection weights/bias on the host
    (softmax rows sum to 1). gamma_q/beta_q (if nontrivial) are one
    per-partition tensor_scalar on q^.
"""
import sys

sys.path.insert(0, "/opt/trn_rl_repo")

from contextlib import ExitStack

import numpy as np

import concourse.bass as bass
import concourse.tile as tile
from concourse import bacc, mybir
from concourse.bass_utils import run_bass_kernel_spmd
from concourse.masks import make_identity

F32 = mybir.dt.float32
F32R = mybir.dt.float32r
F16 = mybir.dt.float16
I32 = mybir.dt.int32
C = 128          # channels
NH = 8           # heads
DH = 16          # head dim
T = 216          # tokens per window (6*6*6)
TC = 108         # tokens per chunk (3 d-slices)
NCORES = 8
EPS = 1e-5

_BUILD_CACHE = {}


def _build_nc(trivial_q: bool, trivial_bias: bool, DEBUG=False):
    key = (trivial_q, trivial_bias, DEBUG)
    if key in _BUILD_CACHE:
        return _BUILD_CACHE[key]

    nc = bacc.Bacc("TRN2", target_bir_lowering=False, debug=False,
                   num_devices=NCORES)
    qs = nc.dram_tensor("q_slab", [C, 8, 8, T], F32, kind="ExternalInput")
    ks = nc.dram_tensor("k_slab", [C, 8, 8, T], F32, kind="ExternalInput")
    vs = nc.dram_tensor("v_slab", [C, 8, 8, T], F32, kind="ExternalInput")
    wt0 = nc.dram_tensor("wt0", [C, C], F16, kind="ExternalInput")
    wt1 = nc.dram_tensor("wt1", [C, C], F16, kind="ExternalInput")
    pb = nc.dram_tensor("pbias", [C, 1], F32, kind="ExternalInput")
    gq = bq = None
    if not trivial_q:
        gq = nc.dram_tensor("gq", [C, 1], F32, kind="ExternalInput")
        bq = nc.dram_tensor("bq", [C, 1], F32, kind="ExternalInput")
    ys = nc.dram_tensor("y_slab", [C, 8, 8, T], F32, kind="ExternalOutput")
    dbg = {}
    if DEBUG:
        dbg["st"] = nc.dram_tensor("dbg_st", [TC, 2, 8, 6], F32, kind="ExternalOutput")
        dbg["rstd"] = nc.dram_tensor("dbg_rstd", [TC, 2, 8, 3], F32, kind="ExternalOutput")
        dbg["D"] = nc.dram_tensor("dbg_D", [TC, 2, 8, 4], F16, kind="ExternalOutput")
        dbg["qh"] = nc.dram_tensor("dbg_qh", [C, T], F16, kind="ExternalOutput")
        dbg["kh"] = nc.dram_tensor("dbg_kh", [C, T], F16, kind="ExternalOutput")
        dbg["qhH"] = nc.dram_tensor("dbg_qhH", [DH, NH, T], F16, kind="ExternalOutput")
        dbg["s4"] = nc.dram_tensor("dbg_s4", [1, 4, 2, TC], F16, kind="ExternalOutput")
        dbg["spb"] = nc.dram_tensor("dbg_spb", [4, 2, TC], F16, kind="ExternalOutput")
        dbg["tv"] = nc.dram_tensor("dbg_tv", [TC, 2, 2, 4, 32], F16, kind="ExternalOutput")
        dbg["E"] = nc.dram_tensor("dbg_E", [TC, 2, 2, 4, T], F16, kind="ExternalOutput")
        dbg["avn"] = nc.dram_tensor("dbg_avn", [C, 2, T], F16, kind="ExternalOutput")
        dbg["R"] = nc.dram_tensor("dbg_R", [C, 2, T], F32, kind="ExternalOutput")

    AF = mybir.ActivationFunctionType
    OP = mybir.AluOpType

    with tile.TileContext(nc) as tc, ExitStack() as ctx:
        consts = ctx.enter_context(tc.tile_pool(name="consts", bufs=1))
        inp = ctx.enter_context(tc.tile_pool(name="inp", bufs=2))
        outp = ctx.enter_context(tc.tile_pool(name="outp", bufs=2))
        sqp = ctx.enter_context(tc.tile_pool(name="sqp", bufs=2))
        qkp = ctx.enter_context(tc.tile_pool(name="qkp", bufs=2))
        qkh = ctx.enter_context(tc.tile_pool(name="qkh", bufs=8))
        tmpp = ctx.enter_context(tc.tile_pool(name="tmpp", bufs=2))
        sS = ctx.enter_context(tc.tile_pool(name="sS", bufs=2))
        ep = ctx.enter_context(tc.tile_pool(name="ep", bufs=2))
        avn_p = ctx.enter_context(tc.tile_pool(name="avn_p", bufs=2))
        rp = ctx.enter_context(tc.tile_pool(name="rp", bufs=2))
        # PSUM (8 banks x 2KB): scr 1 + bcn/bp 2 + big(sc,y) 2x2 + av 1
        p_scr = ctx.enter_context(tc.tile_pool(name="p_scr", bufs=1, space="PSUM"))
        p_bcn = ctx.enter_context(tc.tile_pool(name="p_bcn", bufs=1, space="PSUM"))
        p_big = ctx.enter_context(tc.tile_pool(name="p_big", bufs=2, space="PSUM"))
        p_av = ctx.enter_context(tc.tile_pool(name="p_av", bufs=1, space="PSUM"))

        ident = consts.tile([C, C], F32)
        make_identity(nc, ident[:])
        ident16 = consts.tile([C, C], F16)
        nc.vector.tensor_copy(ident16[:], ident[:])
        ones_col = consts.tile([C, 1], F32)
        nc.vector.memset(ones_col[:], 1.0)
        onesr16 = consts.tile([1, C], F16)
        nc.vector.memset(onesr16[:], 1.0)
        onesr32 = consts.tile([1, 32], F32)
        nc.vector.memset(onesr32[:], 1.0)
        t_wt = []
        for hh, w_dram in enumerate((wt0, wt1)):
            t_w = consts.tile([C, C], F16, tag=f"wt{hh}")
            nc.sync.dma_start(t_w[:], w_dram[:, :])
            t_wt.append(t_w)
        t_pb = None
        if not trivial_bias:
            t_pb = consts.tile([C, 1], F32)
            nc.sync.dma_start(t_pb[:], pb[:, :])
        t_gq = t_bq = None
        if not trivial_q:
            t_gq = consts.tile([C, 1], F32)
            t_bq = consts.tile([C, 1], F32)
            nc.sync.dma_start(t_gq[:], gq[:, :])
            nc.sync.dma_start(t_bq[:], bq[:, :])
        # Derived LN stats (fp16), one persistent tile, written per eighth.
        # Cols: (rq, rk, wq, wk); transposed per window so the 4 stat rows
        # land on partitions 0..3 (a partition-strided DMA gather silently
        # reads the wrong partitions, so keep everything contiguous).
        t_D = consts.tile([TC, 2, 8, 4], F16)
        # f32 per-token v stats (tensor_scalar scalars must be f32):
        # col 0 = rv, col 1 = mv
        t_Dv = consts.tile([TC, 2, 8, 2], F32)
        # v^ stationary tiles: [chunk, hh, g, 32]; col 0 = ones (Z row),
        # cols 1..16 = channels of head 4*hh+g, rest zero.
        t_vh = []
        for w in range(8):
            tv = consts.tile([TC, 2, 2, 4, 32], F16, tag=f"tv{w}", name=f"tv{w}")
            nc.vector.memset(tv[:], 0.0)
            nc.vector.memset(tv[:, :, :, :, 0:1], 1.0)
            t_vh.append(tv)

        def rsqrt_cols(var_view, out_view, scale):
            """out = scale/sqrt(var); bit-trick seed + 2 Newton (no ACT
            table, keeps the activation table pinned on Exp)."""
            p = TC
            ve = sS.tile([p, 2, 8, 3], F32, tag="rs_ve")
            nc.vector.tensor_copy(ve[:], var_view)
            ti = sS.tile([p, 2, 8, 3], I32, tag="rs_ti")
            nc.vector.tensor_scalar(
                ti[:], ve[:].bitcast(I32), 1, None,
                op0=OP.logical_shift_right)
            nc.vector.tensor_scalar(
                ti[:], ti[:], 0, None, op0=OP.bitwise_not)
            nc.vector.tensor_scalar(
                ti[:], ti[:], 0x5f3759df + 1, None, op0=OP.add)
            y_cur = ti[:].bitcast(F32)
            t1 = sS.tile([p, 2, 8, 3], F32, tag="rs_t1")
            for it in range(2):
                nc.vector.tensor_tensor(t1[:], y_cur, y_cur, op=OP.mult)
                nc.vector.tensor_tensor(t1[:], t1[:], ve[:], op=OP.mult)
                nc.vector.tensor_scalar(t1[:], t1[:], -0.5, 1.5,
                                        op0=OP.mult, op1=OP.add)
                if it == 0:
                    yn = sS.tile([p, 2, 8, 3], F32, tag="rs_yn")
                    nc.vector.tensor_tensor(yn[:], t1[:], y_cur, op=OP.mult)
                    y_cur = yn[:]
                else:
                    # fold the final scale into the last Newton multiply
                    nc.vector.tensor_tensor(t1[:], t1[:], y_cur, op=OP.mult)
                    nc.vector.tensor_scalar(out_view, t1[:], scale, None,
                                            op0=OP.mult)

        for ihw in range(8):
            t_q = inp.tile([C, 8, T], F32, tag="in_q")
            t_k = inp.tile([C, 8, T], F32, tag="in_k")
            t_v = inp.tile([C, 8, T], F32, tag="in_v")
            nc.sync.dma_start(t_q[:], qs[:, ihw, :, :])
            nc.sync.dma_start(t_k[:], ks[:, ihw, :, :])
            nc.sync.dma_start(t_v[:], vs[:, ihw, :, :])
            t_oe = outp.tile([C, 8, T], F32)

            # ---- stats: 12 tiny matmuls per window ------------------------
            # st cols: (q_s1, q_s2, k_s1, k_s2, v_s1, v_s2)
            st = p_scr.tile([TC, 2, 8, 6], F32, tag="scr")
            for iww in range(8):
                for it, srcT in enumerate((t_q, t_k, t_v)):
                    win = srcT[:, iww, :]
                    sq = sqp.tile([C, T], F32, tag=f"sq{it}")
                    if it == 2:
                        nc.scalar.square(sq[:], win)
                    else:
                        nc.vector.tensor_tensor(sq[:], win, win, op=OP.mult)
                    for jc in range(2):
                        nc.tensor.matmul(st[:, jc, iww, 2 * it:2 * it + 1],
                                         srcT[:, iww, TC * jc:TC * jc + TC],
                                         ones_col[:],
                                         start=True, stop=True)
                        nc.tensor.matmul(st[:, jc, iww, 2 * it + 1:2 * it + 2],
                                         sq[:, TC * jc:TC * jc + TC],
                                         ones_col[:],
                                         start=True, stop=True)

            # ---- derived stats (batched over the eighth) ------------------
            stS = sS.tile([TC, 2, 8, 6], F32, tag="stS")
            nc.vector.tensor_copy(stS[:], st[:])
            if DEBUG and ihw == 0:
                nc.sync.dma_start(dbg["st"][:, :, :, :], stS[:])
            rr = stS[:].rearrange("p a w (b c) -> p a w b c", c=2)
            s1v, s2v = rr[:, :, :, :, 0], rr[:, :, :, :, 1]
            t1 = sS.tile([TC, 2, 8, 3], F32, tag="t1")
            nc.vector.tensor_tensor(t1[:], s1v, s1v, op=OP.mult)
            u0 = sS.tile([TC, 2, 8, 3], F32, tag="u0")
            nc.vector.tensor_scalar(u0[:], s2v, 128.0, 128.0 * 128.0 * EPS,
                                    op0=OP.mult, op1=OP.add)
            nc.vector.tensor_tensor(u0[:], u0[:], t1[:], op=OP.subtract)
            # rstd = 128/sqrt(u0): qk pair -> D cols {0,1}, v -> Dv col 0
            rstd = sS.tile([TC, 2, 8, 3], F32, tag="rstd")
            rsqrt_cols(u0[:], rstd[:], 128.0)
            nc.vector.tensor_copy(t_D[:, :, :, 0:2], rstd[:, :, :, 0:2])
            nc.vector.tensor_copy(t_Dv[:, :, :, 0:1], rstd[:, :, :, 2:3])
            # means scaled by 1/128: (mq, mk) -> tmp, mv -> D col 2
            m3 = sS.tile([TC, 2, 8, 3], F32, tag="m3")
            nc.vector.tensor_scalar(m3[:], s1v, 1.0 / 128.0, None, op0=OP.mult)
            nc.vector.tensor_copy(t_Dv[:, :, :, 1:2], m3[:, :, :, 2:3])
            # w = mean * rstd -> D cols {2, 3}
            nc.vector.tensor_tensor(t_D[:, :, :, 2:4], m3[:, :, :, 0:2],
                                    rstd[:, :, :, 0:2], op=OP.mult)
            if DEBUG and ihw == 0:
                nc.sync.dma_start(dbg["rstd"][:, :, :, :], rstd[:])
                nc.sync.dma_start(dbg["D"][:, :, :, :], t_D[:])

            # ---- NORM phase: all 8 windows ---------------------------------
            qkH_t = [None] * 8
            for iww in range(8):
                # stat row transpose: [TC, 4] -> [4, 2, TC] (fp16)
                sp = p_scr.tile([4, 2, TC], F16, tag="scr")
                for jc in range(2):
                    nc.tensor.transpose(sp[:, jc, :], t_D[:, jc, iww, :],
                                        ident16[:TC, :TC])
                # stage to SBUF (DMA cannot read PSUM), then fold the
                # 4 stat rows (rq, rk, wq, wk) onto partition 0
                spb = sS.tile([4, 2, TC], F16, tag="spb")
                nc.vector.tensor_copy(spb[:], sp[:])
                s4 = sS.tile([1, 4, 2, TC], F16, tag="s4")
                nc.sync.dma_start(s4[0:1, :, :, :], spb[0:4, :, :])
                # broadcast rows: bcn slices (0=rq, 1=wq, 2=rk, 3=wk)
                bcn = p_bcn.tile([C, 4, 256], F32, tag="bcn")
                for s, dst in ((0, 0), (2, 1), (1, 2), (3, 3)):
                    nc.tensor.matmul(bcn[:, dst, 0:T], onesr16[:],
                                     s4[0:1, s, :, :], start=True, stop=True)
                # q^ = q * rq_b - wq_b (fp16), DVE
                q_win = t_q[:, iww, :]
                k_win = t_k[:, iww, :]
                tmq = tmpp.tile([C, T], F32, tag="tmq")
                nc.vector.tensor_tensor(tmq[:], q_win, bcn[:, 0, 0:T],
                                        op=OP.mult)
                qkn = qkp.tile([C, 2, T], F16, tag="qkn")
                nc.vector.tensor_tensor(qkn[:, 0, :], tmq[:], bcn[:, 1, 0:T],
                                        op=OP.subtract)
                if not trivial_q:
                    nc.vector.tensor_scalar(qkn[:, 0, :], qkn[:, 0, :],
                                            t_gq[:, 0:1], t_bq[:, 0:1],
                                            op0=OP.mult, op1=OP.add)
                tmk = tmpp.tile([C, T], F32, tag="tmk")
                nc.vector.tensor_tensor(tmk[:], k_win, bcn[:, 2, 0:T],
                                        op=OP.mult)
                nc.vector.tensor_tensor(qkn[:, 1, :], tmk[:], bcn[:, 3, 0:T],
                                        op=OP.subtract)
                # restage head-major (q and k together): [16, 8, 2, 216]
                qkH = qkh.tile([DH, NH, 2, T], F16, tag="qkH")
                for h in range(NH):
                    nc.sync.dma_start(qkH[:, h, :, :],
                                      qkn[DH * h:DH * h + DH, :, :])
                qkH_t[iww] = qkH
                if DEBUG and ihw == 0 and iww == 0:
                    nc.sync.dma_start(dbg["qh"][:, :], qkn[:, 0, :])
                    nc.sync.dma_start(dbg["kh"][:, :], qkn[:, 1, :])
                    nc.sync.dma_start(dbg["qhH"][:, :, :], qkH[:, :, 0, :])
                    nc.sync.dma_start(dbg["s4"][:, :, :, :], s4[:])
                    nc.sync.dma_start(dbg["spb"][:, :, :], spb[:])
                # v^: transpose to token-major, normalize per-partition
                vt = p_scr.tile([TC, 2, C], F32, tag="scr")
                for jc in range(2):
                    v_chunk = t_v[:, iww, TC * jc:TC * jc + TC]
                    nc.tensor.transpose(vt[:, jc, :], v_chunk, ident[:])
                tv = t_vh[iww]
                for jc in range(2):
                    vsrc = vt[:, jc, :].rearrange("p (r g d) -> p r g d",
                                                  r=2, g=4)
                    nc.vector.tensor_scalar(
                        tv[:, jc, :, :, 1:17], vsrc,
                        t_Dv[:, jc, iww, 1:2], t_Dv[:, jc, iww, 0:1],
                        op0=OP.subtract, op1=OP.mult)

            # ---- ATTN phase: scores(w) overlapped with tail(w-1) ----------
            E_t = [None] * 8
            for step in range(9):
                if step < 8:
                    iww = step
                    qkH = qkH_t[iww]
                    t_E = ep.tile([TC, 2, 2, 4, T], F16, tag="E")
                    E_t[iww] = t_E
                    for jc in range(2):
                        for hh in range(2):
                            sc = p_big.tile([C, 4, 256], F32, tag="big")
                            for g in range(4):
                                h = 4 * hh + g
                                nc.tensor.matmul(
                                    sc[0:TC, g, 0:T],
                                    qkH[:, h, 1, TC * jc:TC * jc + TC],
                                    qkH[:, h, 0, :], start=True, stop=True)
                            nc.scalar.activation(t_E[:, jc, hh, :, :],
                                                 sc[0:TC, :, 0:T],
                                                 AF.Exp, scale=0.25)
                if step >= 1:
                    iww = step - 1
                    t_E = E_t[iww]
                    tv = t_vh[iww]
                    if DEBUG and ihw == 0 and iww == 0:
                        nc.sync.dma_start(dbg["tv"][:, :, :, :, :], t_vh[0][:])
                        nc.sync.dma_start(dbg["E"][:, :, :, :, :], t_E[:])
                    # attn @ v; ones col makes row 32g the Z row
                    av = p_av.tile([C, 2, T], F32, tag="av")
                    for hh in range(2):
                        for g in range(4):
                            for jc in range(2):
                                nc.tensor.matmul(
                                    av[32 * g:32 * g + 32, hh, :],
                                    tv[:, jc, hh, g, :],
                                    t_E[:, jc, hh, g, :],
                                    start=(jc == 0), stop=(jc == 1),
                                    tile_position=(0, 32 * g))
                    # 1/Z -> fp16 -> gather Z rows to partition 0 -> broadcast
                    t_R = rp.tile([C, 2, T], F32, tag="R")
                    nc.vector.reciprocal_approx_fast(
                        t_R[:].rearrange("p a b -> p (a b)"),
                        av[:].rearrange("p a b -> p (a b)"))
                    t_Rh = rp.tile([C, 2, T], F16, tag="Rh")
                    nc.scalar.copy(t_Rh[:], t_R[:])
                    r4 = rp.tile([1, 4, 2, T], F16, tag="r4")
                    for g in range(4):
                        nc.sync.dma_start(r4[0:1, g, :, :],
                                          t_Rh[32 * g:32 * g + 1, :, :])
                    bp = p_bcn.tile([C, 4, 256], F32, tag="bcn")
                    bpf = bp[:].rearrange("p a b -> p (a b)")
                    for g in range(4):
                        nc.tensor.matmul(
                            bpf[32 * g:32 * g + 32, 0:2 * T],
                            onesr16[0:1, 0:32],
                            r4[0:1, g, :, :].rearrange("p a b -> p (a b)"),
                            start=True, stop=True, tile_position=(0, 32 * g))
                    # TT cannot read two PSUM operands and divide is not a
                    # valid TT op: stage av to SBUF (ACT), then multiply by
                    # the 1/Z broadcast (DVE)
                    avS = avn_p.tile([C, 2, T], F16, tag="avS")
                    nc.scalar.copy(avS[:], av[:])
                    avn = avn_p.tile([C, 2, T], F16, tag="avn")
                    bpv = bpf[:, 0:2 * T].rearrange("p (a b) -> p a b", b=T)
                    nc.vector.tensor_tensor(avn[:], avS[:], bpv, op=OP.mult)
                    if DEBUG and ihw == 0 and iww == 0:
                        nc.sync.dma_start(dbg["avn"][:, :, :], avn[:])
                        nc.sync.dma_start(dbg["R"][:, :, :], t_R[:])
                    # projection
                    y = p_big.tile([C, 4, 256], F32, tag="big")
                    nc.tensor.matmul(y[:, 0, 0:T], t_wt[0][:], avn[:, 0, :],
                                     start=True, stop=False)
                    nc.tensor.matmul(y[:, 0, 0:T], t_wt[1][:], avn[:, 1, :],
                                     start=False, stop=True)
                    out_view = t_oe[:, iww, :]
                    if trivial_bias:
                        nc.scalar.copy(out_view, y[:, 0, 0:T])
                    else:
                        nc.vector.tensor_scalar(out_view, y[:, 0, 0:T],
                                                t_pb[:, 0:1], None, op0=OP.add)

            nc.sync.dma_start(ys[:, ihw, :, :], t_oe[:])

    nc.compile()
    _BUILD_CACHE[key] = nc
    return nc


def _prepare(inputs):
    q_map = np.asarray(inputs["q_map"], np.float32)
    k_map = np.asarray(inputs["k_map"], np.float32)
    v_map = np.asarray(inputs["v_map"], np.float32)
    gamma_q = np.asarray(inputs["gamma_q"], np.float32)
    beta_q = np.asarray(inputs["beta_q"], np.float32)
    gamma_kv = np.asarray(inputs["gamma_kv"], np.float32)
    beta_kv = np.asarray(inputs["beta_kv"], np.float32)
    proj_w = np.asarray(inputs["proj_w"], np.float32)
    proj_b = np.asarray(inputs["proj_b"], np.float32)

    trivial_q = bool(np.all(gamma_q == 1.0) and np.all(beta_q == 0.0))
    trivial_kv = bool(np.all(gamma_kv == 1.0) and np.all(beta_kv == 0.0))
    if not trivial_kv:
        raise NotImplementedError(
            "nontrivial gamma_kv/beta_kv on k not implemented")

    # gamma_kv folds into the projection weight columns; beta_kv adds
    # proj_w @ beta_kv to every output (softmax rows sum to 1).
    wt_v = proj_w.T * gamma_kv[:, None]   # [c_in, c_out]
    bias = proj_b + proj_w @ beta_kv
    trivial_bias = bool(np.all(bias == 0.0))

    # packed av-row layout: row 32g+1+d (hh slot) = channel 16*(4hh+g)+d
    wt0 = np.zeros((C, C), np.float32)
    wt1 = np.zeros((C, C), np.float32)
    for g in range(4):
        for d in range(DH):
            wt0[32 * g + 1 + d] = wt_v[DH * (4 * 0 + g) + d]
            wt1[32 * g + 1 + d] = wt_v[DH * (4 * 1 + g) + d]

    def to_windows(x, m):
        # [C, 6, 48, 48] -> [C, 8hw, 8ww, 216] with token order (d, h, w)
        s = x[0, :, 6 * m:6 * m + 6]
        s = s.reshape(C, 6, 8, 6, 8, 6)
        s = np.transpose(s, (0, 2, 4, 1, 3, 5))
        return np.ascontiguousarray(s.reshape(C, 8, 8, T))

    in_maps = []
    for m in range(NCORES):
        im = {
            "q_slab": to_windows(q_map, m),
            "k_slab": to_windows(k_map, m),
            "v_slab": to_windows(v_map, m),
            "wt0": wt0.astype(np.float16),
            "wt1": wt1.astype(np.float16),
            "pbias": np.ascontiguousarray(bias.reshape(C, 1)),
        }
        if not trivial_q:
            im["gq"] = np.ascontiguousarray(gamma_q.reshape(C, 1))
            im["bq"] = np.ascontiguousarray(beta_q.reshape(C, 1))
        in_maps.append(im)
    return (trivial_q, trivial_bias), in_maps


def _run(inputs, trace=False, **trace_kwargs):
    flags, in_maps = _prepare(inputs)
    nc = _build_nc(*flags)
    res = run_bass_kernel_spmd(nc, in_maps, list(range(NCORES)),
                               trace=trace, **trace_kwargs)
    slabs = []
    for m in range(NCORES):
        s = res.results[m]["y_slab"].reshape(C, 8, 8, 6, 6, 6)
        s = np.transpose(s, (0, 3, 1, 4, 2, 5)).reshape(C, 6, 48, 48)
        slabs.append(s)
    out = np.concatenate(slabs, axis=1).reshape(1, C, 48, 48, 48)
    return out.astype(np.float32), res


def kernel(**inputs):
    out, _ = _run(inputs, trace=False)
    return out


def kernel_traced(**inputs):
    return _run(inputs, trace=True)


# revision 29
# speedup vs baseline: 2.5358x; 1.0068x over previous
"""Trainium2 Bass kernel for windowed 3D cross-attention (CrossAttention3D).

Reference computation:
  (B=1, C=128, D=H=W=48) q/k/v maps -> channels-last -> partition into
  6x6x6 windows (512 windows, 216 tokens each) -> LayerNorm over C ->
  8-head attention within each window (dh=16) -> output projection ->
  window reverse -> (B, C, D, H, W).

Sharding: data-parallel over the window depth axis. Core m processes the
D-slab d in [6m, 6m+6) -- 64 independent windows per core.

v2 design (see commit history for the all-fp32 baseline):
  - PE matmul cost is out_free_size x cycles_per_row and cycles_per_row
    is 4 for fp32 but 1 for fp16 -- all attention matmuls use fp16
    operands (PSUM accumulates fp32).
  - LN stats come from tiny K=128/N=1 PE matmuls in channel-major
    layout (lhsT = x chunk, rhs = ones column) -- no per-window
    transposes of q/k. Normalization applies broadcast stat rows
    (K=1 outer products into PSUM).
  - q^/k^ are DMA-restaged into head-major [16, 8, 216] tiles so every
    score matmul reads partition-base-0 operands (PE operands off base
    0/32/64 are rejected, and mixing tile row-positions within a PSUM
    bank faults at runtime).
  - v^ is PE-transposed to token-major (needed as the attn@v stationary
    operand anyway) and normalized with per-partition scalars; a
    constant ones column yields the softmax denominator Z as row 32g of
    the attn@v PSUM block.
  - exp on the Activation engine straight out of score PSUM; 1/Z rows
    are DMA-gathered to partition 0, broadcast with K=1 outer products,
    and applied in a single tensor_tensor.
  - gamma_kv/beta_kv fold into the projection weights/bias on the host
    (softmax rows sum to 1). gamma_q/beta_q (if nontrivial) are one
    per-partition tensor_scalar on q^.
"""
import sys

sys.path.insert(0, "/opt/trn_rl_repo")

from contextlib import ExitStack

import numpy as np

import concourse.bass as bass
import concourse.tile as tile
from concourse import bacc, mybir
from concourse.bass_utils import run_bass_kernel_spmd
from concourse.masks import make_identity

F32 = mybir.dt.float32
F32R = mybir.dt.float32r
F16 = mybir.dt.float16
I32 = mybir.dt.int32
C = 128          # channels
NH = 8           # heads
DH = 16          # head dim
T = 216          # tokens per window (6*6*6)
TC = 108         # tokens per chunk (3 d-slices)
NCORES = 8
EPS = 1e-5

_BUILD_CACHE = {}


def _build_nc(trivial_q: bool, trivial_bias: bool, DEBUG=False):
    key = (trivial_q, trivial_bias, DEBUG)
    if key in _BUILD_CACHE:
        return _BUILD_CACHE[key]

    nc = bacc.Bacc("TRN2", target_bir_lowering=False, debug=False,
                   num_devices=NCORES)
    qs = nc.dram_tensor("q_slab", [C, 8, 8, T], F32, kind="ExternalInput")
    ks = nc.dram_tensor("k_slab", [C, 8, 8, T], F32, kind="ExternalInput")
    vs = nc.dram_tensor("v_slab", [C, 8, 8, T], F32, kind="ExternalInput")
    wt0 = nc.dram_tensor("wt0", [C, C], F16, kind="ExternalInput")
    wt1 = nc.dram_tensor("wt1", [C, C], F16, kind="ExternalInput")
    pb = nc.dram_tensor("pbias", [C, 1], F32, kind="ExternalInput")
    gq = bq = None
    if not trivial_q:
        gq = nc.dram_tensor("gq", [C, 1], F32, kind="ExternalInput")
        bq = nc.dram_tensor("bq", [C, 1], F32, kind="ExternalInput")
    ys = nc.dram_tensor("y_slab", [C, 8, 8, T], F32, kind="ExternalOutput")
    dbg = {}
    if DEBUG:
        dbg["st"] = nc.dram_tensor("dbg_st", [TC, 2, 8, 6], F32, kind="ExternalOutput")
        dbg["rstd"] = nc.dram_tensor("dbg_rstd", [TC, 2, 8, 3], F32, kind="ExternalOutput")
        dbg["D"] = nc.dram_tensor("dbg_D", [TC, 2, 8, 4], F16, kind="ExternalOutput")
        dbg["qh"] = nc.dram_tensor("dbg_qh", [C, T], F16, kind="ExternalOutput")
        dbg["kh"] = nc.dram_tensor("dbg_kh", [C, T], F16, kind="ExternalOutput")
        dbg["qhH"] = nc.dram_tensor("dbg_qhH", [DH, NH, T], F16, kind="ExternalOutput")
        dbg["s4"] = nc.dram_tensor("dbg_s4", [1, 8 * TC], F16, kind="ExternalOutput")
        dbg["spb"] = nc.dram_tensor("dbg_spb", [8, TC], F16, kind="ExternalOutput")
        dbg["tv"] = nc.dram_tensor("dbg_tv", [TC, 2, 2, 4, 32], F16, kind="ExternalOutput")
        dbg["E"] = nc.dram_tensor("dbg_E", [TC, 2, 2, 4, T], F16, kind="ExternalOutput")
        dbg["avn"] = nc.dram_tensor("dbg_avn", [C, 2, T], F16, kind="ExternalOutput")
        dbg["R"] = nc.dram_tensor("dbg_R", [C, 2, T], F32, kind="ExternalOutput")

    AF = mybir.ActivationFunctionType
    OP = mybir.AluOpType

    with tile.TileContext(nc) as tc, ExitStack() as ctx:
        consts = ctx.enter_context(tc.tile_pool(name="consts", bufs=1))
        inp = ctx.enter_context(tc.tile_pool(name="inp", bufs=2))
        outp = ctx.enter_context(tc.tile_pool(name="outp", bufs=2))
        sqp = ctx.enter_context(tc.tile_pool(name="sqp", bufs=2))
        qkp = ctx.enter_context(tc.tile_pool(name="qkp", bufs=2))
        qkh = ctx.enter_context(tc.tile_pool(name="qkh", bufs=8))
        tmpp = ctx.enter_context(tc.tile_pool(name="tmpp", bufs=2))
        sS = ctx.enter_context(tc.tile_pool(name="sS", bufs=2))
        ep = ctx.enter_context(tc.tile_pool(name="ep", bufs=2))
        avn_p = ctx.enter_context(tc.tile_pool(name="avn_p", bufs=2))
        rp = ctx.enter_context(tc.tile_pool(name="rp", bufs=2))
        # PSUM (8 banks x 2KB): scr 1 + bcn/bp 2 + big(sc,y) 2x2 + av 1
        p_scr = ctx.enter_context(tc.tile_pool(name="p_scr", bufs=1, space="PSUM"))
        p_bcn = ctx.enter_context(tc.tile_pool(name="p_bcn", bufs=1, space="PSUM"))
        p_big = ctx.enter_context(tc.tile_pool(name="p_big", bufs=2, space="PSUM"))
        p_av = ctx.enter_context(tc.tile_pool(name="p_av", bufs=1, space="PSUM"))

        ident = consts.tile([C, C], F32)
        make_identity(nc, ident[:])
        ident16 = consts.tile([C, C], F16)
        nc.vector.tensor_copy(ident16[:], ident[:])
        ones_col = consts.tile([C, 1], F32)
        nc.vector.memset(ones_col[:], 1.0)
        onesr16 = consts.tile([1, C], F16)
        nc.vector.memset(onesr16[:], 1.0)
        onesr32 = consts.tile([1, 32], F32)
        nc.vector.memset(onesr32[:], 1.0)
        t_wt = []
        for hh, w_dram in enumerate((wt0, wt1)):
            t_w = consts.tile([C, C], F16, tag=f"wt{hh}")
            nc.sync.dma_start(t_w[:], w_dram[:, :])
            t_wt.append(t_w)
        t_pb = None
        if not trivial_bias:
            t_pb = consts.tile([C, 1], F32)
            nc.sync.dma_start(t_pb[:], pb[:, :])
        t_gq = t_bq = None
        if not trivial_q:
            t_gq = consts.tile([C, 1], F32)
            t_bq = consts.tile([C, 1], F32)
            nc.sync.dma_start(t_gq[:], gq[:, :])
            nc.sync.dma_start(t_bq[:], bq[:, :])
        # Derived LN stats (fp16), one persistent tile, written per eighth.
        # Cols: (rq, rk, wq, wk); transposed per window so the 4 stat rows
        # land on partitions 0..3 (a partition-strided DMA gather silently
        # reads the wrong partitions, so keep everything contiguous).
        t_D = consts.tile([TC, 8, 4, 2], F16)
        # f32 per-token v stats (tensor_scalar scalars must be f32):
        # col 0 = rv, col 1 = mv
        t_Dv = consts.tile([TC, 2, 8, 2], F32)
        # v^ stationary tiles: [chunk, hh, g, 32]; col 0 = ones (Z row),
        # cols 1..16 = channels of head 4*hh+g, rest zero.
        t_vh = []
        for w in range(8):
            tv = consts.tile([TC, 2, 2, 4, 32], F16, tag=f"tv{w}", name=f"tv{w}")
            nc.vector.memset(tv[:], 0.0)
            nc.vector.memset(tv[:, :, :, :, 0:1], 1.0)
            t_vh.append(tv)

        def rsqrt_cols(var_view, out_view, scale):
            """out = scale/sqrt(var); bit-trick seed + 2 Newton (no ACT
            table, keeps the activation table pinned on Exp)."""
            p = TC
            ve = sS.tile([p, 2, 8, 3], F32, tag="rs_ve")
            nc.vector.tensor_copy(ve[:], var_view)
            ti = sS.tile([p, 2, 8, 3], I32, tag="rs_ti")
            nc.vector.tensor_scalar(
                ti[:], ve[:].bitcast(I32), 1, None,
                op0=OP.logical_shift_right)
            nc.vector.tensor_scalar(
                ti[:], ti[:], 0, None, op0=OP.bitwise_not)
            nc.vector.tensor_scalar(
                ti[:], ti[:], 0x5f3759df + 1, None, op0=OP.add)
            y_cur = ti[:].bitcast(F32)
            t1 = sS.tile([p, 2, 8, 3], F32, tag="rs_t1")
            for it in range(2):
                nc.vector.tensor_tensor(t1[:], y_cur, y_cur, op=OP.mult)
                nc.vector.tensor_tensor(t1[:], t1[:], ve[:], op=OP.mult)
                nc.vector.tensor_scalar(t1[:], t1[:], -0.5, 1.5,
                                        op0=OP.mult, op1=OP.add)
                if it == 0:
                    yn = sS.tile([p, 2, 8, 3], F32, tag="rs_yn")
                    nc.vector.tensor_tensor(yn[:], t1[:], y_cur, op=OP.mult)
                    y_cur = yn[:]
                else:
                    # fold the final scale into the last Newton multiply
                    nc.vector.tensor_tensor(t1[:], t1[:], y_cur, op=OP.mult)
                    nc.vector.tensor_scalar(out_view, t1[:], scale, None,
                                            op0=OP.mult)

        for ihw in range(8):
            t_q = inp.tile([C, 8, T], F32, tag="in_q")
            t_k = inp.tile([C, 8, T], F32, tag="in_k")
            t_v = inp.tile([C, 8, T], F32, tag="in_v")
            nc.sync.dma_start(t_q[:], qs[:, ihw, :, :])
            nc.sync.dma_start(t_k[:], ks[:, ihw, :, :])
            nc.sync.dma_start(t_v[:], vs[:, ihw, :, :])
            t_oe = outp.tile([C, 8, T], F32)

            # ---- stats: 12 tiny matmuls per window ------------------------
            # st cols: (q_s1, q_s2, k_s1, k_s2, v_s1, v_s2)
            st = p_scr.tile([TC, 2, 8, 6], F32, tag="scr")
            for iww in range(8):
                for it, srcT in enumerate((t_q, t_k, t_v)):
                    win = srcT[:, iww, :]
                    sq = sqp.tile([C, T], F32, tag=f"sq{it}")
                    if it == 2:
                        nc.scalar.square(sq[:], win)
                    else:
                        nc.vector.tensor_tensor(sq[:], win, win, op=OP.mult)
                    for jc in range(2):
                        nc.tensor.matmul(st[:, jc, iww, 2 * it:2 * it + 1],
                                         srcT[:, iww, TC * jc:TC * jc + TC],
                                         ones_col[:],
                                         start=True, stop=True)
                        nc.tensor.matmul(st[:, jc, iww, 2 * it + 1:2 * it + 2],
                                         sq[:, TC * jc:TC * jc + TC],
                                         ones_col[:],
                                         start=True, stop=True)

            # ---- derived stats (batched over the eighth) ------------------
            stS = sS.tile([TC, 2, 8, 6], F32, tag="stS")
            nc.vector.tensor_copy(stS[:], st[:])
            if DEBUG and ihw == 0:
                nc.sync.dma_start(dbg["st"][:, :, :, :], stS[:])
            rr = stS[:].rearrange("p a w (b c) -> p a w b c", c=2)
            s1v, s2v = rr[:, :, :, :, 0], rr[:, :, :, :, 1]
            t1 = sS.tile([TC, 2, 8, 3], F32, tag="t1")
            nc.vector.tensor_tensor(t1[:], s1v, s1v, op=OP.mult)
            u0 = sS.tile([TC, 2, 8, 3], F32, tag="u0")
            nc.vector.tensor_scalar(u0[:], s2v, 128.0, 128.0 * 128.0 * EPS,
                                    op0=OP.mult, op1=OP.add)
            nc.vector.tensor_tensor(u0[:], u0[:], t1[:], op=OP.subtract)
            # rstd = 128/sqrt(u0): q -> D stat 0, k -> D stat 2, v -> Dv col 0
            rstd = sS.tile([TC, 2, 8, 3], F32, tag="rstd")
            rsqrt_cols(u0[:], rstd[:], 128.0)
            # D layout [TC, w, (r q, w q, r k, w k), jc]
            Dr = t_D[:].rearrange("p w (x y) a -> p w x y a", y=2)
            nc.vector.tensor_copy(Dr[:, :, :, 0, :],
                                  rstd[:, :, :, 0:2].rearrange("p a w c -> p w c a"))
            nc.vector.tensor_copy(t_Dv[:, :, :, 0:1], rstd[:, :, :, 2:3])
            # means scaled by 1/128: (mq, mk) -> tmp, mv -> D col 2
            m3 = sS.tile([TC, 2, 8, 3], F32, tag="m3")
            nc.vector.tensor_scalar(m3[:], s1v, 1.0 / 128.0, None, op0=OP.mult)
            nc.vector.tensor_copy(t_Dv[:, :, :, 1:2], m3[:, :, :, 2:3])
            # w = mean * rstd -> D stats {1, 3}
            nc.vector.tensor_tensor(
                Dr[:, :, :, 1, :],
                m3[:, :, :, 0:2].rearrange("p a w c -> p w c a"),
                rstd[:, :, :, 0:2].rearrange("p a w c -> p w c a"), op=OP.mult)
            if DEBUG and ihw == 0:
                nc.sync.dma_start(dbg["rstd"][:, :, :, :], rstd[:])
                nc.sync.dma_start(dbg["D"][:, :, :, :], t_D[:])

            # ---- NORM phase: all 8 windows ---------------------------------
            qkH_t = [None] * 8
            for iww in range(8):
                # stat row transpose, (stat, jc)-ordered rows: in free dims
                # permuted to (c, a) so out row = 2*s + jc; D cols are
                # (rq, wq, rk, wk) so rows 0-3 = q stats, 4-7 = k stats
                sp = p_scr.tile([8, TC], F16, tag="scr")
                nc.tensor.transpose(sp[:, :], t_D[:, iww, :, :],
                                    ident16[:TC, :TC])
                # stage to SBUF (DMA cannot read PSUM), fold onto partition 0
                spb = sS.tile([8, TC], F16, tag="spb")
                nc.vector.tensor_copy(spb[:], sp[:])
                s4 = sS.tile([1, 8 * TC], F16, tag="s4")
                nc.sync.dma_start(
                    s4[0:1, :].rearrange("p (s t) -> p s t", s=8), spb[0:8, :])
                # broadcast: bcn[:, i, :] = (r 216 | w 216) for tensor i
                bcn = p_bcn.tile([C, 2, 512], F32, tag="bcn")
                for i in range(2):
                    nc.tensor.matmul(
                        bcn[:, i, 0:432],
                        onesr16[:], s4[0:1, 432 * i:432 * i + 432],
                        start=True, stop=True)
                # q^ = q * rq_b - wq_b (fp16), DVE
                q_win = t_q[:, iww, :]
                k_win = t_k[:, iww, :]
                tmq = tmpp.tile([C, T], F32, tag="tmq")
                nc.vector.tensor_tensor(tmq[:], q_win, bcn[:, 0, 0:T],
                                        op=OP.mult)
                qkn = qkp.tile([C, 2, T], F16, tag="qkn")
                nc.vector.tensor_tensor(qkn[:, 0, :], tmq[:], bcn[:, 0, T:2 * T],
                                        op=OP.subtract)
                if not trivial_q:
                    nc.vector.tensor_scalar(qkn[:, 0, :], qkn[:, 0, :],
                                            t_gq[:, 0:1], t_bq[:, 0:1],
                                            op0=OP.mult, op1=OP.add)
                tmk = tmpp.tile([C, T], F32, tag="tmk")
                nc.vector.tensor_tensor(tmk[:], k_win, bcn[:, 1, 0:T],
                                        op=OP.mult)
                nc.vector.tensor_tensor(qkn[:, 1, :], tmk[:], bcn[:, 1, T:2 * T],
                                        op=OP.subtract)
                # restage head-major (q and k together): [16, 8, 2, 216]
                qkH = qkh.tile([DH, NH, 2, T], F16, tag="qkH")
                for h in range(NH):
                    nc.sync.dma_start(qkH[:, h, :, :],
                                      qkn[DH * h:DH * h + DH, :, :])
                qkH_t[iww] = qkH
                if DEBUG and ihw == 0 and iww == 0:
                    nc.sync.dma_start(dbg["qh"][:, :], qkn[:, 0, :])
                    nc.sync.dma_start(dbg["kh"][:, :], qkn[:, 1, :])
                    nc.sync.dma_start(dbg["qhH"][:, :, :], qkH[:, :, 0, :])
                    nc.sync.dma_start(dbg["s4"][:, :], s4[:])
                    nc.sync.dma_start(dbg["spb"][:, :], spb[:])
                # v^: transpose to token-major, normalize per-partition
                vt = p_scr.tile([TC, 2, C], F32, tag="scr")
                for jc in range(2):
                    v_chunk = t_v[:, iww, TC * jc:TC * jc + TC]
                    nc.tensor.transpose(vt[:, jc, :], v_chunk, ident[:])
                tv = t_vh[iww]
                for jc in range(2):
                    vsrc = vt[:, jc, :].rearrange("p (r g d) -> p r g d",
                                                  r=2, g=4)
                    nc.vector.tensor_scalar(
                        tv[:, jc, :, :, 1:17], vsrc,
                        t_Dv[:, jc, iww, 1:2], t_Dv[:, jc, iww, 0:1],
                        op0=OP.subtract, op1=OP.mult)

            # ---- ATTN phase: scores(w) overlapped with tail(w-1) ----------
            E_t = [None] * 8
            for step in range(9):
                if step < 8:
                    iww = step
                    qkH = qkH_t[iww]
                    t_E = ep.tile([TC, 2, 2, 4, T], F16, tag="E")
                    E_t[iww] = t_E
                    for jc in range(2):
                        for hh in range(2):
                            sc = p_big.tile([C, 4, 256], F32, tag="big")
                            for g in range(4):
                                h = 4 * hh + g
                                nc.tensor.matmul(
                                    sc[0:TC, g, 0:T],
                                    qkH[:, h, 1, TC * jc:TC * jc + TC],
                                    qkH[:, h, 0, :], start=True, stop=True)
                            nc.scalar.activation(t_E[:, jc, hh, :, :],
                                                 sc[0:TC, :, 0:T],
                                                 AF.Exp, scale=0.25)
                if step >= 1:
                    iww = step - 1
                    t_E = E_t[iww]
                    tv = t_vh[iww]
                    if DEBUG and ihw == 0 and iww == 0:
                        nc.sync.dma_start(dbg["tv"][:, :, :, :, :], t_vh[0][:])
                        nc.sync.dma_start(dbg["E"][:, :, :, :, :], t_E[:])
                    # attn @ v; ones col makes row 32g the Z row
                    av = p_av.tile([C, 2, T], F32, tag="av")
                    for hh in range(2):
                        for g in range(4):
                            for jc in range(2):
                                nc.tensor.matmul(
                                    av[32 * g:32 * g + 32, hh, :],
                                    tv[:, jc, hh, g, :],
                                    t_E[:, jc, hh, g, :],
                                    start=(jc == 0), stop=(jc == 1),
                                    tile_position=(0, 32 * g))
                    # 1/Z -> fp16 -> gather Z rows to partition 0 -> broadcast
                    t_R = rp.tile([C, 2, T], F32, tag="R")
                    nc.vector.reciprocal_approx_fast(
                        t_R[:].rearrange("p a b -> p (a b)"),
                        av[:].rearrange("p a b -> p (a b)"))
                    t_Rh = rp.tile([C, 2, T], F16, tag="Rh")
                    nc.scalar.copy(t_Rh[:], t_R[:])
                    r4 = rp.tile([1, 4, 2, T], F16, tag="r4")
                    for g in range(4):
                        nc.sync.dma_start(r4[0:1, g, :, :],
                                          t_Rh[32 * g:32 * g + 1, :, :])
                    bp = p_bcn.tile([C, 2, 512], F32, tag="bcn")
                    bpf = bp[:].rearrange("p a b -> p (a b)")
                    for g in range(4):
                        nc.tensor.matmul(
                            bpf[32 * g:32 * g + 32, 0:2 * T],
                            onesr16[0:1, 0:32],
                            r4[0:1, g, :, :].rearrange("p a b -> p (a b)"),
                            start=True, stop=True, tile_position=(0, 32 * g))
                    # TT cannot read two PSUM operands and divide is not a
                    # valid TT op: stage av to SBUF (ACT), then multiply by
                    # the 1/Z broadcast (DVE)
                    avS = avn_p.tile([C, 2, T], F16, tag="avS")
                    nc.scalar.copy(avS[:], av[:])
                    avn = avn_p.tile([C, 2, T], F16, tag="avn")
                    bpv = bpf[:, 0:2 * T].rearrange("p (a b) -> p a b", b=T)
                    nc.vector.tensor_tensor(avn[:], avS[:], bpv, op=OP.mult)
                    if DEBUG and ihw == 0 and iww == 0:
                        nc.sync.dma_start(dbg["avn"][:, :, :], avn[:])
                        nc.sync.dma_start(dbg["R"][:, :, :], t_R[:])
                    # projection
                    y = p_big.tile([C, 4, 256], F32, tag="big")
                    nc.tensor.matmul(y[:, 0, 0:T], t_wt[0][:], avn[:, 0, :],
                                     start=True, stop=False)
                    nc.tensor.matmul(y[:, 0, 0:T], t_wt[1][:], avn[:, 1, :],
                                     start=False, stop=True)
                    out_view = t_oe[:, iww, :]
                    if trivial_bias:
                        nc.scalar.copy(out_view, y[:, 0, 0:T])
                    else:
                        nc.vector.tensor_scalar(out_view, y[:, 0, 0:T],
                                                t_pb[:, 0:1], None, op0=OP.add)

            nc.sync.dma_start(ys[:, ihw, :, :], t_oe[:])

    nc.compile()
    _BUILD_CACHE[key] = nc
    return nc


def _prepare(inputs):
    q_map = np.asarray(inputs["q_map"], np.float32)
    k_map = np.asarray(inputs["k_map"], np.float32)
    v_map = np.asarray(inputs["v_map"], np.float32)
    gamma_q = np.asarray(inputs["gamma_q"], np.float32)
    beta_q = np.asarray(inputs["beta_q"], np.float32)
    gamma_kv = np.asarray(inputs["gamma_kv"], np.float32)
    beta_kv = np.asarray(inputs["beta_kv"], np.float32)
    proj_w = np.asarray(inputs["proj_w"], np.float32)
    proj_b = np.asarray(inputs["proj_b"], np.float32)

    trivial_q = bool(np.all(gamma_q == 1.0) and np.all(beta_q == 0.0))
    trivial_kv = bool(np.all(gamma_kv == 1.0) and np.all(beta_kv == 0.0))
    if not trivial_kv:
        raise NotImplementedError(
            "nontrivial gamma_kv/beta_kv on k not implemented")

    # gamma_kv folds into the projection weight columns; beta_kv adds
    # proj_w @ beta_kv to every output (softmax rows sum to 1).
    wt_v = proj_w.T * gamma_kv[:, None]   # [c_in, c_out]
    bias = proj_b + proj_w @ beta_kv
    trivial_bias = bool(np.all(bias == 0.0))

    # packed av-row layout: row 32g+1+d (hh slot) = channel 16*(4hh+g)+d
    wt0 = np.zeros((C, C), np.float32)
    wt1 = np.zeros((C, C), np.float32)
    for g in range(4):
        for d in range(DH):
            wt0[32 * g + 1 + d] = wt_v[DH * (4 * 0 + g) + d]
            wt1[32 * g + 1 + d] = wt_v[DH * (4 * 1 + g) + d]

    def to_windows(x, m):
        # [C, 6, 48, 48] -> [C, 8hw, 8ww, 216] with token order (d, h, w)
        s = x[0, :, 6 * m:6 * m + 6]
        s = s.reshape(C, 6, 8, 6, 8, 6)
        s = np.transpose(s, (0, 2, 4, 1, 3, 5))
        return np.ascontiguousarray(s.reshape(C, 8, 8, T))

    in_maps = []
    for m in range(NCORES):
        im = {
            "q_slab": to_windows(q_map, m),
            "k_slab": to_windows(k_map, m),
            "v_slab": to_windows(v_map, m),
            "wt0": wt0.astype(np.float16),
            "wt1": wt1.astype(np.float16),
            "pbias": np.ascontiguousarray(bias.reshape(C, 1)),
        }
        if not trivial_q:
            im["gq"] = np.ascontiguousarray(gamma_q.reshape(C, 1))
            im["bq"] = np.ascontiguousarray(beta_q.reshape(C, 1))
        in_maps.append(im)
    return (trivial_q, trivial_bias), in_maps


def _run(inputs, trace=False, **trace_kwargs):
    flags, in_maps = _prepare(inputs)
    nc = _build_nc(*flags)
    res = run_bass_kernel_spmd(nc, in_maps, list(range(NCORES)),
                               trace=trace, **trace_kwargs)
    slabs = []
    for m in range(NCORES):
        s = res.results[m]["y_slab"].reshape(C, 8, 8, 6, 6, 6)
        s = np.transpose(s, (0, 3, 1, 4, 2, 5)).reshape(C, 6, 48, 48)
        slabs.append(s)
    out = np.concatenate(slabs, axis=1).reshape(1, C, 48, 48, 48)
    return out.astype(np.float32), res


def kernel(**inputs):
    out, _ = _run(inputs, trace=False)
    return out


def kernel_traced(**inputs):
    return _run(inputs, trace=True)


# revision 31
# speedup vs baseline: 2.5374x; 1.0006x over previous
"""Trainium2 Bass kernel for windowed 3D cross-attention (CrossAttention3D).

Reference computation:
  (B=1, C=128, D=H=W=48) q/k/v maps -> channels-last -> partition into
  6x6x6 windows (512 windows, 216 tokens each) -> LayerNorm over C ->
  8-head attention within each window (dh=16) -> output projection ->
  window reverse -> (B, C, D, H, W).

Sharding: data-parallel over the window depth axis. Core m processes the
D-slab d in [6m, 6m+6) -- 64 independent windows per core.

v2 design (see commit history for the all-fp32 baseline):
  - PE matmul cost is out_free_size x cycles_per_row and cycles_per_row
    is 4 for fp32 but 1 for fp16 -- all attention matmuls use fp16
    operands (PSUM accumulates fp32).
  - LN stats come from tiny K=128/N=1 PE matmuls in channel-major
    layout (lhsT = x chunk, rhs = ones column) -- no per-window
    transposes of q/k. Normalization applies broadcast stat rows
    (K=1 outer products into PSUM).
  - q^/k^ are DMA-restaged into head-major [16, 8, 216] tiles so every
    score matmul reads partition-base-0 operands (PE operands off base
    0/32/64 are rejected, and mixing tile row-positions within a PSUM
    bank faults at runtime).
  - v^ is PE-transposed to token-major (needed as the attn@v stationary
    operand anyway) and normalized with per-partition scalars; a
    constant ones column yields the softmax denominator Z as row 32g of
    the attn@v PSUM block.
  - exp on the Activation engine straight out of score PSUM; 1/Z rows
    are DMA-gathered to partition 0, broadcast with K=1 outer products,
    and applied in a single tensor_tensor.
  - gamma_kv/beta_kv fold into the projection weights/bias on the host
    (softmax rows sum to 1). gamma_q/beta_q (if nontrivial) are one
    per-partition tensor_scalar on q^.
"""
import sys

sys.path.insert(0, "/opt/trn_rl_repo")

from contextlib import ExitStack

import numpy as np

import concourse.bass as bass
import concourse.tile as tile
from concourse import bacc, mybir
from concourse.bass_utils import run_bass_kernel_spmd
from concourse.masks import make_identity

F32 = mybir.dt.float32
F32R = mybir.dt.float32r
F16 = mybir.dt.float16
I32 = mybir.dt.int32
C = 128          # channels
NH = 8           # heads
DH = 16          # head dim
T = 216          # tokens per window (6*6*6)
TC = 108         # tokens per chunk (3 d-slices)
NCORES = 8
EPS = 1e-5

_BUILD_CACHE = {}


def _build_nc(trivial_q: bool, trivial_bias: bool, DEBUG=False):
    key = (trivial_q, trivial_bias, DEBUG)
    if key in _BUILD_CACHE:
        return _BUILD_CACHE[key]

    nc = bacc.Bacc("TRN2", target_bir_lowering=False, debug=False,
                   num_devices=NCORES)
    qs = nc.dram_tensor("q_slab", [C, 8, 8, T], F32, kind="ExternalInput")
    ks = nc.dram_tensor("k_slab", [C, 8, 8, T], F32, kind="ExternalInput")
    vs = nc.dram_tensor("v_slab", [C, 8, 8, T], F32, kind="ExternalInput")
    wt0 = nc.dram_tensor("wt0", [C, C], F16, kind="ExternalInput")
    wt1 = nc.dram_tensor("wt1", [C, C], F16, kind="ExternalInput")
    pb = nc.dram_tensor("pbias", [C, 1], F32, kind="ExternalInput")
    gq = bq = None
    if not trivial_q:
        gq = nc.dram_tensor("gq", [C, 1], F32, kind="ExternalInput")
        bq = nc.dram_tensor("bq", [C, 1], F32, kind="ExternalInput")
    ys = nc.dram_tensor("y_slab", [C, 8, 8, T], F32, kind="ExternalOutput")
    dbg = {}
    if DEBUG:
        dbg["st"] = nc.dram_tensor("dbg_st", [TC, 2, 8, 6], F32, kind="ExternalOutput")
        dbg["rstd"] = nc.dram_tensor("dbg_rstd", [TC, 2, 8, 3], F32, kind="ExternalOutput")
        dbg["D"] = nc.dram_tensor("dbg_D", [TC, 2, 8, 4], F16, kind="ExternalOutput")
        dbg["qh"] = nc.dram_tensor("dbg_qh", [C, T], F16, kind="ExternalOutput")
        dbg["kh"] = nc.dram_tensor("dbg_kh", [C, T], F16, kind="ExternalOutput")
        dbg["qhH"] = nc.dram_tensor("dbg_qhH", [DH, NH, T], F16, kind="ExternalOutput")
        dbg["s4"] = nc.dram_tensor("dbg_s4", [1, 8 * TC], F16, kind="ExternalOutput")
        dbg["spb"] = nc.dram_tensor("dbg_spb", [8, TC], F16, kind="ExternalOutput")
        dbg["tv"] = nc.dram_tensor("dbg_tv", [TC, 2, 2, 4, 32], F16, kind="ExternalOutput")
        dbg["E"] = nc.dram_tensor("dbg_E", [TC, 2, 2, 4, T], F16, kind="ExternalOutput")
        dbg["avn"] = nc.dram_tensor("dbg_avn", [C, 2, T], F16, kind="ExternalOutput")
        dbg["R"] = nc.dram_tensor("dbg_R", [C, 2, T], F32, kind="ExternalOutput")

    AF = mybir.ActivationFunctionType
    OP = mybir.AluOpType

    with tile.TileContext(nc) as tc, ExitStack() as ctx:
        consts = ctx.enter_context(tc.tile_pool(name="consts", bufs=1))
        inp = ctx.enter_context(tc.tile_pool(name="inp", bufs=2))
        outp = ctx.enter_context(tc.tile_pool(name="outp", bufs=2))
        sqp = ctx.enter_context(tc.tile_pool(name="sqp", bufs=2))
        qkp = ctx.enter_context(tc.tile_pool(name="qkp", bufs=2))
        qkh = ctx.enter_context(tc.tile_pool(name="qkh", bufs=8))
        tmpp = ctx.enter_context(tc.tile_pool(name="tmpp", bufs=2))
        sS = ctx.enter_context(tc.tile_pool(name="sS", bufs=2))
        ep = ctx.enter_context(tc.tile_pool(name="ep", bufs=2))
        avn_p = ctx.enter_context(tc.tile_pool(name="avn_p", bufs=2))
        rp = ctx.enter_context(tc.tile_pool(name="rp", bufs=2))
        # PSUM (8 banks x 2KB): scr 1 + bcn/bp 2 + big(sc,y) 2x2 + av 1
        p_scr = ctx.enter_context(tc.tile_pool(name="p_scr", bufs=1, space="PSUM"))
        p_bcn = ctx.enter_context(tc.tile_pool(name="p_bcn", bufs=1, space="PSUM"))
        p_big = ctx.enter_context(tc.tile_pool(name="p_big", bufs=2, space="PSUM"))
        p_av = ctx.enter_context(tc.tile_pool(name="p_av", bufs=1, space="PSUM"))

        ident = consts.tile([C, C], F32)
        make_identity(nc, ident[:])
        ident16 = consts.tile([C, C], F16)
        nc.vector.tensor_copy(ident16[:], ident[:])
        ones_col = consts.tile([C, 1], F32)
        nc.vector.memset(ones_col[:], 1.0)
        onesr16 = consts.tile([1, C], F16)
        nc.vector.memset(onesr16[:], 1.0)
        onesr32 = consts.tile([1, 32], F32)
        nc.vector.memset(onesr32[:], 1.0)
        t_wt = []
        for hh, w_dram in enumerate((wt0, wt1)):
            t_w = consts.tile([C, C], F16, tag=f"wt{hh}")
            nc.sync.dma_start(t_w[:], w_dram[:, :])
            t_wt.append(t_w)
        t_pb = None
        if not trivial_bias:
            t_pb = consts.tile([C, 1], F32)
            nc.sync.dma_start(t_pb[:], pb[:, :])
        t_gq = t_bq = None
        if not trivial_q:
            t_gq = consts.tile([C, 1], F32)
            t_bq = consts.tile([C, 1], F32)
            nc.sync.dma_start(t_gq[:], gq[:, :])
            nc.sync.dma_start(t_bq[:], bq[:, :])
        # Derived LN stats (fp16), one persistent tile, written per eighth.
        # Cols: (rq, rk, wq, wk); transposed per window so the 4 stat rows
        # land on partitions 0..3 (a partition-strided DMA gather silently
        # reads the wrong partitions, so keep everything contiguous).
        t_D = consts.tile([TC, 8, 4, 2], F16)
        # f32 per-token v stats (tensor_scalar scalars must be f32):
        # col 0 = rv, col 1 = mv
        t_Dv = consts.tile([TC, 2, 8, 2], F32)
        # v^ stationary tiles: [chunk, hh, g, 32]; col 0 = ones (Z row),
        # cols 1..16 = channels of head 4*hh+g, rest zero.
        t_vh = []
        for w in range(8):
            tv = consts.tile([TC, 2, 2, 4, 32], F16, tag=f"tv{w}", name=f"tv{w}")
            nc.vector.memset(tv[:], 0.0)
            nc.vector.memset(tv[:, :, :, :, 0:1], 1.0)
            t_vh.append(tv)

        def rsqrt_cols(var_view, out_view, scale):
            """out = scale/sqrt(var); bit-trick seed + 2 Newton (no ACT
            table, keeps the activation table pinned on Exp)."""
            p = TC
            ve = sS.tile([p, 2, 8, 3], F32, tag="rs_ve")
            nc.vector.tensor_copy(ve[:], var_view)
            ti = sS.tile([p, 2, 8, 3], I32, tag="rs_ti")
            nc.vector.tensor_scalar(
                ti[:], ve[:].bitcast(I32), 1, None,
                op0=OP.logical_shift_right)
            nc.vector.tensor_scalar(
                ti[:], ti[:], 0, None, op0=OP.bitwise_not)
            nc.vector.tensor_scalar(
                ti[:], ti[:], 0x5f3759df + 1, None, op0=OP.add)
            y_cur = ti[:].bitcast(F32)
            t1 = sS.tile([p, 2, 8, 3], F32, tag="rs_t1")
            for it in range(2):
                nc.vector.tensor_tensor(t1[:], y_cur, y_cur, op=OP.mult)
                nc.vector.tensor_tensor(t1[:], t1[:], ve[:], op=OP.mult)
                nc.vector.tensor_scalar(t1[:], t1[:], -0.5, 1.5,
                                        op0=OP.mult, op1=OP.add)
                if it == 0:
                    yn = sS.tile([p, 2, 8, 3], F32, tag="rs_yn")
                    nc.vector.tensor_tensor(yn[:], t1[:], y_cur, op=OP.mult)
                    y_cur = yn[:]
                else:
                    # fold the final scale into the last Newton multiply
                    nc.vector.tensor_tensor(t1[:], t1[:], y_cur, op=OP.mult)
                    nc.vector.tensor_scalar(out_view, t1[:], scale, None,
                                            op0=OP.mult)

        for ihw in range(8):
            t_q = inp.tile([C, 8, T], F32, tag="in_q")
            t_k = inp.tile([C, 8, T], F32, tag="in_k")
            t_v = inp.tile([C, 8, T], F32, tag="in_v")
            nc.sync.dma_start(t_q[:], qs[:, ihw, :, :])
            nc.sync.dma_start(t_k[:], ks[:, ihw, :, :])
            nc.sync.dma_start(t_v[:], vs[:, ihw, :, :])
            t_oe = outp.tile([C, 8, T], F32)

            # ---- stats: 12 tiny matmuls per window ------------------------
            # st cols: (q_s1, q_s2, k_s1, k_s2, v_s1, v_s2)
            st = p_scr.tile([TC, 2, 8, 6], F32, tag="scr")
            for iww in range(8):
                for it, srcT in enumerate((t_q, t_k, t_v)):
                    win = srcT[:, iww, :]
                    sq = sqp.tile([C, T], F32, tag=f"sq{it}")
                    if it == 2:
                        nc.scalar.square(sq[:], win)
                    else:
                        nc.vector.tensor_tensor(sq[:], win, win, op=OP.mult)
                    for jc in range(2):
                        nc.tensor.matmul(st[:, jc, iww, 2 * it:2 * it + 1],
                                         srcT[:, iww, TC * jc:TC * jc + TC],
                                         ones_col[:],
                                         start=True, stop=True)
                        nc.tensor.matmul(st[:, jc, iww, 2 * it + 1:2 * it + 2],
                                         sq[:, TC * jc:TC * jc + TC],
                                         ones_col[:],
                                         start=True, stop=True)

            # ---- derived stats (batched over the eighth) ------------------
            stS = sS.tile([TC, 2, 8, 6], F32, tag="stS")
            nc.vector.tensor_copy(stS[:], st[:])
            if DEBUG and ihw == 0:
                nc.sync.dma_start(dbg["st"][:, :, :, :], stS[:])
            rr = stS[:].rearrange("p a w (b c) -> p a w b c", c=2)
            s1v, s2v = rr[:, :, :, :, 0], rr[:, :, :, :, 1]
            t1 = sS.tile([TC, 2, 8, 3], F32, tag="t1")
            nc.vector.tensor_tensor(t1[:], s1v, s1v, op=OP.mult)
            u0 = sS.tile([TC, 2, 8, 3], F32, tag="u0")
            nc.vector.tensor_scalar(u0[:], s2v, 128.0, 128.0 * 128.0 * EPS,
                                    op0=OP.mult, op1=OP.add)
            nc.vector.tensor_tensor(u0[:], u0[:], t1[:], op=OP.subtract)
            # rstd = 128/sqrt(u0): q -> D stat 0, k -> D stat 2, v -> Dv col 0
            rstd = sS.tile([TC, 2, 8, 3], F32, tag="rstd")
            rsqrt_cols(u0[:], rstd[:], 128.0)
            # D layout [TC, w, (r q, w q, r k, w k), jc]
            Dr = t_D[:].rearrange("p w (x y) a -> p w x y a", y=2)
            nc.vector.tensor_copy(Dr[:, :, :, 0, :],
                                  rstd[:, :, :, 0:2].rearrange("p a w c -> p w c a"))
            nc.vector.tensor_copy(t_Dv[:, :, :, 0:1], rstd[:, :, :, 2:3])
            # means scaled by 1/128: (mq, mk) -> tmp, mv -> D col 2
            m3 = sS.tile([TC, 2, 8, 3], F32, tag="m3")
            nc.vector.tensor_scalar(m3[:], s1v, 1.0 / 128.0, None, op0=OP.mult)
            nc.vector.tensor_copy(t_Dv[:, :, :, 1:2], m3[:, :, :, 2:3])
            # w = mean * rstd -> D stats {1, 3}
            nc.vector.tensor_tensor(
                Dr[:, :, :, 1, :],
                m3[:, :, :, 0:2].rearrange("p a w c -> p w c a"),
                rstd[:, :, :, 0:2].rearrange("p a w c -> p w c a"), op=OP.mult)
            if DEBUG and ihw == 0:
                nc.sync.dma_start(dbg["rstd"][:, :, :, :], rstd[:])
                nc.sync.dma_start(dbg["D"][:, :, :, :], t_D[:])

            # ---- NORM phase: all 8 windows ---------------------------------
            qkH_t = [None] * 8
            for iww in range(8):
                # stat row transpose, (stat, jc)-ordered rows: in free dims
                # permuted to (c, a) so out row = 2*s + jc; D cols are
                # (rq, wq, rk, wk) so rows 0-3 = q stats, 4-7 = k stats
                sp = p_scr.tile([8, TC], F16, tag="scr")
                nc.tensor.transpose(sp[:, :], t_D[:, iww, :, :],
                                    ident16[:TC, :TC])
                # stage to SBUF (DMA cannot read PSUM), fold onto partition 0
                spb = sS.tile([8, TC], F16, tag="spb")
                nc.vector.tensor_copy(spb[:], sp[:])
                s4 = sS.tile([1, 8 * TC], F16, tag="s4")
                nc.sync.dma_start(
                    s4[0:1, :].rearrange("p (s t) -> p s t", s=8), spb[0:8, :])
                # broadcast: bcn[:, i, :] = (r 216 | w 216) for tensor i
                bcn = p_bcn.tile([C, 2, 512], F32, tag="bcn")
                for i in range(2):
                    nc.tensor.matmul(
                        bcn[:, i, 0:432],
                        onesr16[:], s4[0:1, 432 * i:432 * i + 432],
                        start=True, stop=True)
                # q^ = q * rq_b - wq_b (fp16), DVE
                q_win = t_q[:, iww, :]
                k_win = t_k[:, iww, :]
                tmq = tmpp.tile([C, T], F32, tag="tmq")
                nc.vector.tensor_tensor(tmq[:], q_win, bcn[:, 0, 0:T],
                                        op=OP.mult)
                qkn = qkp.tile([C, 2, T], F16, tag="qkn")
                nc.vector.tensor_tensor(qkn[:, 0, :], tmq[:], bcn[:, 0, T:2 * T],
                                        op=OP.subtract)
                if not trivial_q:
                    nc.vector.tensor_scalar(qkn[:, 0, :], qkn[:, 0, :],
                                            t_gq[:, 0:1], t_bq[:, 0:1],
                                            op0=OP.mult, op1=OP.add)
                tmk = tmpp.tile([C, T], F32, tag="tmk")
                nc.vector.tensor_tensor(tmk[:], k_win, bcn[:, 1, 0:T],
                                        op=OP.mult)
                nc.vector.tensor_tensor(qkn[:, 1, :], tmk[:], bcn[:, 1, T:2 * T],
                                        op=OP.subtract)
                # restage head-major (q and k together): [16, 8, 2, 216]
                qkH = qkh.tile([DH, NH, 2, T], F16, tag="qkH")
                for h in range(NH):
                    nc.sync.dma_start(qkH[:, h, :, :],
                                      qkn[DH * h:DH * h + DH, :, :])
                qkH_t[iww] = qkH
                if DEBUG and ihw == 0 and iww == 0:
                    nc.sync.dma_start(dbg["qh"][:, :], qkn[:, 0, :])
                    nc.sync.dma_start(dbg["kh"][:, :], qkn[:, 1, :])
                    nc.sync.dma_start(dbg["qhH"][:, :, :], qkH[:, :, 0, :])
                    nc.sync.dma_start(dbg["s4"][:, :], s4[:])
                    nc.sync.dma_start(dbg["spb"][:, :], spb[:])
                # v^: transpose to token-major, normalize per-partition
                vt = p_scr.tile([TC, 2, C], F32, tag="scr")
                for jc in range(2):
                    v_chunk = t_v[:, iww, TC * jc:TC * jc + TC]
                    nc.tensor.transpose(vt[:, jc, :], v_chunk, ident[:])
                tv = t_vh[iww]
                for jc in range(2):
                    vsrc = vt[:, jc, :].rearrange("p (r g d) -> p r g d",
                                                  r=2, g=4)
                    nc.vector.tensor_scalar(
                        tv[:, jc, :, :, 1:17], vsrc,
                        t_Dv[:, jc, iww, 1:2], t_Dv[:, jc, iww, 0:1],
                        op0=OP.subtract, op1=OP.mult)

            # ---- ATTN phase: scores(w) interleaved with tail(w-1) ---------
            # PE order per step: sc1(w) av0(w-1) sc2(w) av1(w-1) sc3(w)
            # sc4(w) Zout(w-1) proj(w-1) -- exp waits hidden behind av work.
            E_t = [None] * 8
            prev = None
            for step in range(9):
                cur = None
                if step < 8:
                    iww = step
                    qkH = qkH_t[iww]
                    t_E = ep.tile([TC, 2, 2, 4, T], F16, tag="E")
                    E_t[iww] = t_E
                    cur = (iww, t_E)

                def emit_scores(grp):
                    jc, hh = grp // 2, grp % 2
                    iww, t_E = cur
                    qkH = qkH_t[iww]
                    sc = p_big.tile([C, 4, 256], F32, tag="big", name="sc")
                    for g in range(4):
                        h = 4 * hh + g
                        nc.tensor.matmul(
                            sc[0:TC, g, 0:T],
                            qkH[:, h, 1, TC * jc:TC * jc + TC],
                            qkH[:, h, 0, :], start=True, stop=True)
                    nc.scalar.activation(t_E[:, jc, hh, :, :],
                                         sc[0:TC, :, 0:T], AF.Exp, scale=0.25)

                def emit_av(hh):
                    iww, t_E, tv, av = prev[0], prev[1], prev[2], prev[3]
                    for g in range(4):
                        for jc in range(2):
                            nc.tensor.matmul(
                                av[32 * g:32 * g + 32, hh, :],
                                tv[:, jc, hh, g, :],
                                t_E[:, jc, hh, g, :],
                                start=(jc == 0), stop=(jc == 1),
                                tile_position=(0, 32 * g))

                if prev is not None:
                    piww, pE = prev[0], prev[1]
                    av_t = p_av.tile([C, 2, T], F32, tag="av", name="av_t")
                    prev = (piww, pE, t_vh[piww], av_t)
                    if DEBUG and ihw == 0 and piww == 0:
                        nc.sync.dma_start(dbg["tv"][:, :, :, :, :], t_vh[0][:])
                        nc.sync.dma_start(dbg["E"][:, :, :, :, :], pE[:])

                if cur is not None:
                    emit_scores(0)
                if prev is not None:
                    emit_av(0)
                if cur is not None:
                    emit_scores(1)
                if prev is not None:
                    emit_av(1)
                    piww, pE, ptv, av = prev
                    # 1/Z; issue the ACT/DMA legs early so they clear the
                    # queues before Zout needs them
                    t_R = rp.tile([C, 2, T], F32, tag="R")
                    nc.vector.reciprocal_approx_fast(
                        t_R[:].rearrange("p a b -> p (a b)"),
                        av[:].rearrange("p a b -> p (a b)"))
                    t_Rh = rp.tile([C, 2, T], F16, tag="Rh")
                    nc.scalar.copy(t_Rh[:], t_R[:])
                    r4 = rp.tile([1, 4, 2, T], F16, tag="r4")
                    for g in range(4):
                        nc.sync.dma_start(r4[0:1, g, :, :],
                                          t_Rh[32 * g:32 * g + 1, :, :])
                    avS = avn_p.tile([C, 2, T], F16, tag="avS")
                    nc.scalar.copy(avS[:], av[:])
                if cur is not None:
                    emit_scores(2)
                    emit_scores(3)
                if prev is not None:
                    piww, pE, ptv, av = prev
                    bp = p_bcn.tile([C, 2, 512], F32, tag="bcn")
                    bpf = bp[:].rearrange("p a b -> p (a b)")
                    for g in range(4):
                        nc.tensor.matmul(
                            bpf[32 * g:32 * g + 32, 0:2 * T],
                            onesr16[0:1, 0:32],
                            r4[0:1, g, :, :].rearrange("p a b -> p (a b)"),
                            start=True, stop=True, tile_position=(0, 32 * g))
                    avn = avn_p.tile([C, 2, T], F16, tag="avn")
                    bpv = bpf[:, 0:2 * T].rearrange("p (a b) -> p a b", b=T)
                    nc.vector.tensor_tensor(avn[:], avS[:], bpv, op=OP.mult)
                    if DEBUG and ihw == 0 and piww == 0:
                        nc.sync.dma_start(dbg["avn"][:, :, :], avn[:])
                        nc.sync.dma_start(dbg["R"][:, :, :], t_R[:])
                    y = p_big.tile([C, 4, 256], F32, tag="big")
                    nc.tensor.matmul(y[:, 0, 0:T], t_wt[0][:], avn[:, 0, :],
                                     start=True, stop=False)
                    nc.tensor.matmul(y[:, 0, 0:T], t_wt[1][:], avn[:, 1, :],
                                     start=False, stop=True)
                    out_view = t_oe[:, piww, :]
                    if trivial_bias:
                        nc.scalar.copy(out_view, y[:, 0, 0:T])
                    else:
                        nc.vector.tensor_scalar(out_view, y[:, 0, 0:T],
                                                t_pb[:, 0:1], None, op0=OP.add)
                prev = cur

            nc.sync.dma_start(ys[:, ihw, :, :], t_oe[:])

    nc.compile()
    _BUILD_CACHE[key] = nc
    return nc


def _prepare(inputs):
    q_map = np.asarray(inputs["q_map"], np.float32)
    k_map = np.asarray(inputs["k_map"], np.float32)
    v_map = np.asarray(inputs["v_map"], np.float32)
    gamma_q = np.asarray(inputs["gamma_q"], np.float32)
    beta_q = np.asarray(inputs["beta_q"], np.float32)
    gamma_kv = np.asarray(inputs["gamma_kv"], np.float32)
    beta_kv = np.asarray(inputs["beta_kv"], np.float32)
    proj_w = np.asarray(inputs["proj_w"], np.float32)
    proj_b = np.asarray(inputs["proj_b"], np.float32)

    trivial_q = bool(np.all(gamma_q == 1.0) and np.all(beta_q == 0.0))
    trivial_kv = bool(np.all(gamma_kv == 1.0) and np.all(beta_kv == 0.0))
    if not trivial_kv:
        raise NotImplementedError(
            "nontrivial gamma_kv/beta_kv on k not implemented")

    # gamma_kv folds into the projection weight columns; beta_kv adds
    # proj_w @ beta_kv to every output (softmax rows sum to 1).
    wt_v = proj_w.T * gamma_kv[:, None]   # [c_in, c_out]
    bias = proj_b + proj_w @ beta_kv
    trivial_bias = bool(np.all(bias == 0.0))

    # packed av-row layout: row 32g+1+d (hh slot) = channel 16*(4hh+g)+d
    wt0 = np.zeros((C, C), np.float32)
    wt1 = np.zeros((C, C), np.float32)
    for g in range(4):
        for d in range(DH):
            wt0[32 * g + 1 + d] = wt_v[DH * (4 * 0 + g) + d]
            wt1[32 * g + 1 + d] = wt_v[DH * (4 * 1 + g) + d]

    def to_windows(x, m):
        # [C, 6, 48, 48] -> [C, 8hw, 8ww, 216] with token order (d, h, w)
        s = x[0, :, 6 * m:6 * m + 6]
        s = s.reshape(C, 6, 8, 6, 8, 6)
        s = np.transpose(s, (0, 2, 4, 1, 3, 5))
        return np.ascontiguousarray(s.reshape(C, 8, 8, T))

    in_maps = []
    for m in range(NCORES):
        im = {
            "q_slab": to_windows(q_map, m),
            "k_slab": to_windows(k_map, m),
            "v_slab": to_windows(v_map, m),
            "wt0": wt0.astype(np.float16),
            "wt1": wt1.astype(np.float16),
            "pbias": np.ascontiguousarray(bias.reshape(C, 1)),
        }
        if not trivial_q:
            im["gq"] = np.ascontiguousarray(gamma_q.reshape(C, 1))
            im["bq"] = np.ascontiguousarray(beta_q.reshape(C, 1))
        in_maps.append(im)
    return (trivial_q, trivial_bias), in_maps


def _run(inputs, trace=False, **trace_kwargs):
    flags, in_maps = _prepare(inputs)
    nc = _build_nc(*flags)
    res = run_bass_kernel_spmd(nc, in_maps, list(range(NCORES)),
                               trace=trace, **trace_kwargs)
    slabs = []
    for m in range(NCORES):
        s = res.results[m]["y_slab"].reshape(C, 8, 8, 6, 6, 6)
        s = np.transpose(s, (0, 3, 1, 4, 2, 5)).reshape(C, 6, 48, 48)
        slabs.append(s)
    out = np.concatenate(slabs, axis=1).reshape(1, C, 48, 48, 48)
    return out.astype(np.float32), res


def kernel(**inputs):
    out, _ = _run(inputs, trace=False)
    return out


def kernel_traced(**inputs):
    return _run(inputs, trace=True)


# revision 33
# speedup vs baseline: 2.6778x; 1.0553x over previous
"""Trainium2 Bass kernel for windowed 3D cross-attention (CrossAttention3D).

Reference computation:
  (B=1, C=128, D=H=W=48) q/k/v maps -> channels-last -> partition into
  6x6x6 windows (512 windows, 216 tokens each) -> LayerNorm over C ->
  8-head attention within each window (dh=16) -> output projection ->
  window reverse -> (B, C, D, H, W).

Sharding: data-parallel over the window depth axis. Core m processes the
D-slab d in [6m, 6m+6) -- 64 independent windows per core. The host
rewrites each slab window-major ([C, 8, 8, 216]) so every access in the
kernel is contiguous, and un-permutes the output.

Kernel structure: a flat 4-stage software pipeline over the 64 windows,
one iteration per window index W:
  A: LN stats for window W      (x^2 on DVE/ACT + 12 tiny K=128 N=1
                                 PE matmuls; batched derived-stat math
                                 per eighth)
  B: normalization for W-8      (PE-broadcast stat rows; q^/k^ fp16 in
                                 channel-major; v^ PE-transposed to
                                 token-major with a ones column; q^/k^
                                 DMA-restaged head-major so all score
                                 matmuls read partition-base-0 operands)
  C: scores + exp for W-16      (16 fp16 matmuls; exp on ACT from PSUM)
  D: attn@v + softmax-divide + projection for W-17, interleaved with
     C's score groups so the PE never waits on exp.

All heavy matmuls use fp16 operands (1 PE cycle/row vs 4 for fp32).
gamma_kv/beta_kv fold into the projection weights/bias on the host
(softmax rows sum to 1); gamma_q/beta_q (if nontrivial) are one
per-partition tensor_scalar on q^.

Hardware constraints baked in (probed on device): PE operands must sit
at partition base 0/32/64 (explicit off-base tile_position faults);
mixing tile row-positions within one PSUM bank faults; matmul RHS APs
must be single-free-dim; GPSIMD cannot touch PSUM; TT cannot read two
PSUM operands; partition-strided DMA gathers silently read the wrong
partitions (only contiguous partition folds work).
"""
import sys

sys.path.insert(0, "/opt/trn_rl_repo")

from contextlib import ExitStack

import numpy as np

import concourse.bass as bass
import concourse.tile as tile
from concourse import bacc, mybir
from concourse.bass_utils import run_bass_kernel_spmd
from concourse.masks import make_identity

F32 = mybir.dt.float32
F16 = mybir.dt.float16
I32 = mybir.dt.int32
C = 128          # channels
NH = 8           # heads
DH = 16          # head dim
T = 216          # tokens per window (6*6*6)
TC = 108         # tokens per chunk (3 d-slices)
NCORES = 8
EPS = 1e-5
NW = 64          # windows per core

_BUILD_CACHE = {}


def _build_nc(trivial_q: bool, trivial_bias: bool, DEBUG=False):
    key = (trivial_q, trivial_bias, DEBUG)
    if key in _BUILD_CACHE:
        return _BUILD_CACHE[key]

    nc = bacc.Bacc("TRN2", target_bir_lowering=False, debug=False,
                   num_devices=NCORES)
    qs = nc.dram_tensor("q_slab", [C, 8, 8, T], F32, kind="ExternalInput")
    ks = nc.dram_tensor("k_slab", [C, 8, 8, T], F32, kind="ExternalInput")
    vs = nc.dram_tensor("v_slab", [C, 8, 8, T], F32, kind="ExternalInput")
    wt0 = nc.dram_tensor("wt0", [C, C], F16, kind="ExternalInput")
    wt1 = nc.dram_tensor("wt1", [C, C], F16, kind="ExternalInput")
    pb = nc.dram_tensor("pbias", [C, 1], F32, kind="ExternalInput")
    gq = bq = None
    if not trivial_q:
        gq = nc.dram_tensor("gq", [C, 1], F32, kind="ExternalInput")
        bq = nc.dram_tensor("bq", [C, 1], F32, kind="ExternalInput")
    ys = nc.dram_tensor("y_slab", [C, 8, 8, T], F32, kind="ExternalOutput")

    AF = mybir.ActivationFunctionType
    OP = mybir.AluOpType

    with tile.TileContext(nc) as tc, ExitStack() as ctx:
        consts = ctx.enter_context(tc.tile_pool(name="consts", bufs=1))
        inp = ctx.enter_context(tc.tile_pool(name="inp", bufs=3))
        outp = ctx.enter_context(tc.tile_pool(name="outp", bufs=2))
        sqp = ctx.enter_context(tc.tile_pool(name="sqp", bufs=2))
        qkp = ctx.enter_context(tc.tile_pool(name="qkp", bufs=2))
        qkh = ctx.enter_context(tc.tile_pool(name="qkh", bufs=9))
        tmpp = ctx.enter_context(tc.tile_pool(name="tmpp", bufs=2))
        sS = ctx.enter_context(tc.tile_pool(name="sS", bufs=2))
        ep = ctx.enter_context(tc.tile_pool(name="ep", bufs=3))
        avn_p = ctx.enter_context(tc.tile_pool(name="avn_p", bufs=2))
        rp = ctx.enter_context(tc.tile_pool(name="rp", bufs=2))
        # PSUM (8 banks x 2KB):
        # st 1 + scr 1 + bcn 2 + bp 1 + big(sc,y) 2 + av 1 = 8
        p_st = ctx.enter_context(tc.tile_pool(name="p_st", bufs=1, space="PSUM"))
        p_scr = ctx.enter_context(tc.tile_pool(name="p_scr", bufs=1, space="PSUM"))
        p_bcn = ctx.enter_context(tc.tile_pool(name="p_bcn", bufs=1, space="PSUM"))
        p_bp = ctx.enter_context(tc.tile_pool(name="p_bp", bufs=1, space="PSUM"))
        p_big = ctx.enter_context(tc.tile_pool(name="p_big", bufs=1, space="PSUM"))
        p_av = ctx.enter_context(tc.tile_pool(name="p_av", bufs=1, space="PSUM"))

        ident = consts.tile([C, C], F32)
        make_identity(nc, ident[:])
        ident16 = consts.tile([C, C], F16)
        nc.vector.tensor_copy(ident16[:], ident[:])
        ones_col = consts.tile([C, 1], F32)
        nc.vector.memset(ones_col[:], 1.0)
        onesr16 = consts.tile([1, C], F16)
        nc.vector.memset(onesr16[:], 1.0)
        t_wt = []
        for hh, w_dram in enumerate((wt0, wt1)):
            t_w = consts.tile([C, C], F16, tag=f"wt{hh}")
            nc.sync.dma_start(t_w[:], w_dram[:, :])
            t_wt.append(t_w)
        t_pb = None
        if not trivial_bias:
            t_pb = consts.tile([C, 1], F32)
            nc.sync.dma_start(t_pb[:], pb[:, :])
        t_gq = t_bq = None
        if not trivial_q:
            t_gq = consts.tile([C, 1], F32)
            t_bq = consts.tile([C, 1], F32)
            nc.sync.dma_start(t_gq[:], gq[:, :])
            nc.sync.dma_start(t_bq[:], bq[:, :])
        # Derived LN stats (fp16), double-buffered per eighth.
        # Layout [TC, win, (rq, wq, rk, wk), jc]: the (stat, jc) free dims
        # are contiguous so one transpose yields rows 2*s + jc with each
        # tensor's 4 rows adjacent.
        t_D = [consts.tile([TC, 8, 4, 2], F16, tag=f"D{i}", name=f"D{i}")
               for i in range(2)]
        t_Dv = [consts.tile([TC, 2, 8, 2], F32, tag=f"Dv{i}", name=f"Dv{i}")
                for i in range(2)]
        # v^ stationary tiles (ring of 10): [chunk, hh, g, 32];
        # col 0 = ones (Z row), cols 1..16 = channels of head 4*hh+g.
        NTV = 10
        t_vh = []
        for w in range(NTV):
            tv = consts.tile([TC, 2, 2, 4, 32], F16, tag=f"tv{w}", name=f"tv{w}")
            nc.vector.memset(tv[:], 0.0)
            nc.vector.memset(tv[:, :, :, :, 0:1], 1.0)
            t_vh.append(tv)

        def rsqrt_cols(var_view, out_view, scale):
            """out = scale/sqrt(var); bit-trick seed + 2 Newton (no ACT
            table, keeps the activation table pinned on Exp)."""
            p = TC
            ve = sS.tile([p, 2, 8, 3], F32, tag="rs_ve")
            nc.vector.tensor_copy(ve[:], var_view)
            ti = sS.tile([p, 2, 8, 3], I32, tag="rs_ti")
            nc.vector.tensor_scalar(
                ti[:], ve[:].bitcast(I32), 1, None,
                op0=OP.logical_shift_right)
            nc.vector.tensor_scalar(
                ti[:], ti[:], 0, None, op0=OP.bitwise_not)
            nc.vector.tensor_scalar(
                ti[:], ti[:], 0x5f3759df + 1, None, op0=OP.add)
            y_cur = ti[:].bitcast(F32)
            t1 = sS.tile([p, 2, 8, 3], F32, tag="rs_t1")
            for it in range(2):
                nc.vector.tensor_tensor(t1[:], y_cur, y_cur, op=OP.mult)
                nc.vector.tensor_tensor(t1[:], t1[:], ve[:], op=OP.mult)
                nc.vector.tensor_scalar(t1[:], t1[:], -0.5, 1.5,
                                        op0=OP.mult, op1=OP.add)
                if it == 0:
                    yn = sS.tile([p, 2, 8, 3], F32, tag="rs_yn")
                    nc.vector.tensor_tensor(yn[:], t1[:], y_cur, op=OP.mult)
                    y_cur = yn[:]
                else:
                    nc.vector.tensor_tensor(t1[:], t1[:], y_cur, op=OP.mult)
                    nc.vector.tensor_scalar(out_view, t1[:], scale, None,
                                            op0=OP.mult)

        slabs = {}       # eighth -> (t_q, t_k, t_v)
        st_t = {}        # eighth -> stats PSUM tile
        oe_t = {}        # eighth -> output slab tile
        qkH_t = [None] * NW
        E_t = [None] * NW

        def load_slab(e):
            t_q = inp.tile([C, 8, T], F32, tag="in_q", name="t_q")
            t_k = inp.tile([C, 8, T], F32, tag="in_k", name="t_k")
            t_v = inp.tile([C, 8, T], F32, tag="in_v", name="t_v")
            nc.sync.dma_start(t_q[:], qs[:, e, :, :])
            nc.sync.dma_start(t_k[:], ks[:, e, :, :])
            nc.sync.dma_start(t_v[:], vs[:, e, :, :])
            slabs[e] = (t_q, t_k, t_v)

        load_slab(0)

        def stage_A(W):
            e, w = W // 8, W % 8
            if w == 0:
                if e + 1 < 8:
                    load_slab(e + 1)
                st_t[e] = p_st.tile([TC, 2, 8, 6], F32, tag="st", name="st")
            st = st_t[e]
            t_q, t_k, t_v = slabs[e]
            # st cols: (q_s1, q_s2, k_s1, k_s2, v_s1, v_s2)
            for it, srcT in enumerate((t_q, t_k, t_v)):
                win = srcT[:, w, :]
                sq = sqp.tile([C, T], F32, tag=f"sq{it}", name="sq")
                if it == 2:
                    nc.scalar.square(sq[:], win)
                else:
                    nc.vector.tensor_tensor(sq[:], win, win, op=OP.mult)
                for jc in range(2):
                    nc.tensor.matmul(st[:, jc, w, 2 * it:2 * it + 1],
                                     srcT[:, w, TC * jc:TC * jc + TC],
                                     ones_col[:], start=True, stop=True)
                    nc.tensor.matmul(st[:, jc, w, 2 * it + 1:2 * it + 2],
                                     sq[:, TC * jc:TC * jc + TC],
                                     ones_col[:], start=True, stop=True)

        def derived(e):
            st = st_t.pop(e)
            tD, tDv = t_D[e % 2], t_Dv[e % 2]
            stS = sS.tile([TC, 2, 8, 6], F32, tag="stS")
            nc.vector.tensor_copy(stS[:], st[:])
            rr = stS[:].rearrange("p a w (b c) -> p a w b c", c=2)
            s1v, s2v = rr[:, :, :, :, 0], rr[:, :, :, :, 1]
            t1 = sS.tile([TC, 2, 8, 3], F32, tag="t1")
            nc.vector.tensor_tensor(t1[:], s1v, s1v, op=OP.mult)
            u0 = sS.tile([TC, 2, 8, 3], F32, tag="u0")
            nc.vector.tensor_scalar(u0[:], s2v, 128.0, 128.0 * 128.0 * EPS,
                                    op0=OP.mult, op1=OP.add)
            nc.vector.tensor_tensor(u0[:], u0[:], t1[:], op=OP.subtract)
            # rstd = 128/sqrt(u0): q -> D stat 0, k -> D stat 2, v -> Dv 0
            rstd = sS.tile([TC, 2, 8, 3], F32, tag="rstd")
            rsqrt_cols(u0[:], rstd[:], 128.0)
            Dr = tD[:].rearrange("p w (x y) a -> p w x y a", y=2)
            nc.vector.tensor_copy(
                Dr[:, :, :, 0, :],
                rstd[:, :, :, 0:2].rearrange("p a w c -> p w c a"))
            nc.vector.tensor_copy(tDv[:, :, :, 0:1], rstd[:, :, :, 2:3])
            # means scaled by 1/128: (mq, mk) -> tmp, mv -> Dv col 1
            m3 = sS.tile([TC, 2, 8, 3], F32, tag="m3")
            nc.vector.tensor_scalar(m3[:], s1v, 1.0 / 128.0, None, op0=OP.mult)
            nc.vector.tensor_copy(tDv[:, :, :, 1:2], m3[:, :, :, 2:3])
            # w = mean * rstd -> D stats {1, 3}
            nc.vector.tensor_tensor(
                Dr[:, :, :, 1, :],
                m3[:, :, :, 0:2].rearrange("p a w c -> p w c a"),
                rstd[:, :, :, 0:2].rearrange("p a w c -> p w c a"), op=OP.mult)
            if DEBUG and e == 0:
                nc.sync.dma_start(dbg["st"][:, :, :, :], stS[:])
                nc.sync.dma_start(dbg["rstd"][:, :, :, :], rstd[:])
                nc.sync.dma_start(dbg["D"][:, :, :, :], tD[:])

        def stage_B(W):
            e, w = W // 8, W % 8
            tD, tDv = t_D[e % 2], t_Dv[e % 2]
            t_q, t_k, t_v = slabs[e]
            if w == 7:
                del slabs[e]
            # stat row transpose: out row = 2*s + jc, q rows 0-3, k rows 4-7
            sp = p_scr.tile([8, TC], F16, tag="scr", name="sp")
            nc.tensor.transpose(sp[:, :], tD[:, w, :, :], ident16[:TC, :TC])
            spb = sS.tile([8, TC], F16, tag="spb")
            nc.vector.tensor_copy(spb[:], sp[:])
            s4 = sS.tile([1, 8 * TC], F16, tag="s4")
            nc.sync.dma_start(
                s4[0:1, :].rearrange("p (s t) -> p s t", s=8), spb[0:8, :])
            # broadcast: bcn[:, i, 0:216] = r, [:, i, 216:432] = w
            bcn = p_bcn.tile([C, 2, 512], F32, tag="bcn", name="bcn")
            for i in range(2):
                nc.tensor.matmul(bcn[:, i, 0:432], onesr16[:],
                                 s4[0:1, 432 * i:432 * i + 432],
                                 start=True, stop=True)
            # q^ / k^ (fp16, channel-major)
            q_win = t_q[:, w, :]
            k_win = t_k[:, w, :]
            tmq = tmpp.tile([C, T], F32, tag="tmq")
            nc.vector.tensor_tensor(tmq[:], q_win, bcn[:, 0, 0:T], op=OP.mult)
            qkn = qkp.tile([C, 2, T], F16, tag="qkn")
            nc.vector.tensor_tensor(qkn[:, 0, :], tmq[:], bcn[:, 0, T:2 * T],
                                    op=OP.subtract)
            if not trivial_q:
                nc.vector.tensor_scalar(qkn[:, 0, :], qkn[:, 0, :],
                                        t_gq[:, 0:1], t_bq[:, 0:1],
                                        op0=OP.mult, op1=OP.add)
            tmk = tmpp.tile([C, T], F32, tag="tmk")
            nc.vector.tensor_tensor(tmk[:], k_win, bcn[:, 1, 0:T], op=OP.mult)
            nc.vector.tensor_tensor(qkn[:, 1, :], tmk[:], bcn[:, 1, T:2 * T],
                                    op=OP.subtract)
            # restage head-major (q and k together): [16, 8, 2, 216]
            qkH = qkh.tile([DH, NH, 2, T], F16, tag="qkH", name="qkH")
            for h in range(NH):
                nc.sync.dma_start(qkH[:, h, :, :],
                                  qkn[DH * h:DH * h + DH, :, :])
            qkH_t[W] = qkH
            if DEBUG and W == 0:
                nc.sync.dma_start(dbg["qh"][:, :], qkn[:, 0, :])
                nc.sync.dma_start(dbg["kh"][:, :], qkn[:, 1, :])
                nc.sync.dma_start(dbg["qhH"][:, :, :], qkH[:, :, 0, :])
                nc.sync.dma_start(dbg["s4"][:, :], s4[:])
                nc.sync.dma_start(dbg["spb"][:, :], spb[:])
            # v^: transpose to token-major, normalize per-partition
            vt = p_scr.tile([TC, 2, C], F32, tag="scr", name="vt")
            for jc in range(2):
                nc.tensor.transpose(vt[:, jc, :],
                                    t_v[:, w, TC * jc:TC * jc + TC], ident[:])
            tv = t_vh[W % NTV]
            for jc in range(2):
                vsrc = vt[:, jc, :].rearrange("p (r g d) -> p r g d", r=2, g=4)
                nc.vector.tensor_scalar(
                    tv[:, jc, :, :, 1:17], vsrc,
                    tDv[:, jc, w, 1:2], tDv[:, jc, w, 0:1],
                    op0=OP.subtract, op1=OP.mult)

        def scores_group(W, grp):
            jc, hh = grp // 2, grp % 2
            qkH = qkH_t[W]
            t_E = E_t[W]
            sc = p_big.tile([C, 4, 256], F32, tag="big", name="sc")
            for g in range(4):
                h = 4 * hh + g
                nc.tensor.matmul(sc[0:TC, g, 0:T],
                                 qkH[:, h, 1, TC * jc:TC * jc + TC],
                                 qkH[:, h, 0, :], start=True, stop=True)
            nc.scalar.activation(t_E[:, jc, hh, :, :], sc[0:TC, :, 0:T],
                                 AF.Exp, scale=0.25)

        def stage_D(W, part, state):
            t_E = E_t[W]
            tv = t_vh[W % NTV]
            if part == 0:
                av = p_av.tile([C, 2, T], F32, tag="av", name="av")
                state[W] = [av, None, None, None]
                if DEBUG and W == 0:
                    nc.sync.dma_start(dbg["tv"][:, :, :, :, :], tv[:])
                    nc.sync.dma_start(dbg["E"][:, :, :, :, :], t_E[:])
            av = state[W][0]
            if part in (0, 1):
                hh = part
                for g in range(4):
                    for jc in range(2):
                        nc.tensor.matmul(
                            av[32 * g:32 * g + 32, hh, :],
                            tv[:, jc, hh, g, :], t_E[:, jc, hh, g, :],
                            start=(jc == 0), stop=(jc == 1),
                            tile_position=(0, 32 * g))
            if part == 1:
                # 1/Z; issue ACT/DMA legs early so they clear the queues
                t_R = rp.tile([C, 2, T], F32, tag="R")
                nc.vector.reciprocal_approx_fast(
                    t_R[:].rearrange("p a b -> p (a b)"),
                    av[:].rearrange("p a b -> p (a b)"))
                t_Rh = rp.tile([C, 2, T], F16, tag="Rh")
                nc.scalar.copy(t_Rh[:], t_R[:])
                r4 = rp.tile([1, 4, 2, T], F16, tag="r4")
                for g in range(4):
                    nc.sync.dma_start(r4[0:1, g, :, :],
                                      t_Rh[32 * g:32 * g + 1, :, :])
                avS = avn_p.tile([C, 2, T], F16, tag="avS")
                nc.scalar.copy(avS[:], av[:])
                state[W][1:4] = [t_R, r4, avS]
            if part == 2:
                av, t_R, r4, avS = state.pop(W)
                bp = p_bp.tile([C, 512], F32, tag="bp", name="bp")
                for g in range(4):
                    nc.tensor.matmul(
                        bp[32 * g:32 * g + 32, 0:2 * T],
                        onesr16[0:1, 0:32],
                        r4[0:1, g, :, :].rearrange("p a b -> p (a b)"),
                        start=True, stop=True, tile_position=(0, 32 * g))
                avn = avn_p.tile([C, 2, T], F16, tag="avn")
                bpv = bp[:, 0:2 * T].rearrange("p (a b) -> p a b", b=T)
                nc.vector.tensor_tensor(avn[:], avS[:], bpv, op=OP.mult)
                if DEBUG and W == 0:
                    nc.sync.dma_start(dbg["avn"][:, :, :], avn[:])
                    nc.sync.dma_start(dbg["R"][:, :, :], t_R[:])
                y = p_big.tile([C, 4, 256], F32, tag="big", name="y")
                nc.tensor.matmul(y[:, 0, 0:T], t_wt[0][:], avn[:, 0, :],
                                 start=True, stop=False)
                nc.tensor.matmul(y[:, 0, 0:T], t_wt[1][:], avn[:, 1, :],
                                 start=False, stop=True)
                e, w = W // 8, W % 8
                if w == 0:
                    oe_t[e] = outp.tile([C, 8, T], F32, tag="oe", name="oe")
                t_oe = oe_t[e]
                out_view = t_oe[:, w, :]
                if trivial_bias:
                    nc.scalar.copy(out_view, y[:, 0, 0:T])
                else:
                    nc.vector.tensor_scalar(out_view, y[:, 0, 0:T],
                                            t_pb[:, 0:1], None, op0=OP.add)
                if w == 7:
                    nc.sync.dma_start(ys[:, e, :, :], oe_t.pop(e)[:])

        dstate = {}
        for W in range(NW + 17):
            WA, WB, WC, WD = W, W - 8, W - 16, W - 17
            if WA < NW:
                stage_A(WA)
                if WA % 8 == 7:
                    derived(WA // 8)
            if 0 <= WB < NW:
                stage_B(WB)
            if 0 <= WC < NW:
                E_t[WC] = ep.tile([TC, 2, 2, 4, T], F16, tag="E", name="E")
                scores_group(WC, 0)
            if 0 <= WD < NW:
                stage_D(WD, 0, dstate)
            if 0 <= WC < NW:
                scores_group(WC, 1)
            if 0 <= WD < NW:
                stage_D(WD, 1, dstate)
            if 0 <= WC < NW:
                scores_group(WC, 2)
                scores_group(WC, 3)
            if 0 <= WD < NW:
                stage_D(WD, 2, dstate)
            if 0 <= WC < NW - 1:
                # release python refs early (tiles free via pool ring)
                qkH_t[WC - 1 if WC else 0] = None

    nc.compile()
    _BUILD_CACHE[key] = nc
    return nc


def _prepare(inputs):
    q_map = np.asarray(inputs["q_map"], np.float32)
    k_map = np.asarray(inputs["k_map"], np.float32)
    v_map = np.asarray(inputs["v_map"], np.float32)
    gamma_q = np.asarray(inputs["gamma_q"], np.float32)
    beta_q = np.asarray(inputs["beta_q"], np.float32)
    gamma_kv = np.asarray(inputs["gamma_kv"], np.float32)
    beta_kv = np.asarray(inputs["beta_kv"], np.float32)
    proj_w = np.asarray(inputs["proj_w"], np.float32)
    proj_b = np.asarray(inputs["proj_b"], np.float32)

    trivial_q = bool(np.all(gamma_q == 1.0) and np.all(beta_q == 0.0))
    trivial_kv = bool(np.all(gamma_kv == 1.0) and np.all(beta_kv == 0.0))
    if not trivial_kv:
        raise NotImplementedError(
            "nontrivial gamma_kv/beta_kv on k not implemented")

    # gamma_kv folds into the projection weight columns; beta_kv adds
    # proj_w @ beta_kv to every output (softmax rows sum to 1).
    wt_v = proj_w.T * gamma_kv[:, None]   # [c_in, c_out]
    bias = proj_b + proj_w @ beta_kv
    trivial_bias = bool(np.all(bias == 0.0))

    # packed av-row layout: row 32g+1+d (hh slot) = channel 16*(4hh+g)+d
    wt0 = np.zeros((C, C), np.float32)
    wt1 = np.zeros((C, C), np.float32)
    for g in range(4):
        for d in range(DH):
            wt0[32 * g + 1 + d] = wt_v[DH * g + d]
            wt1[32 * g + 1 + d] = wt_v[DH * (4 + g) + d]

    def to_windows(x, m):
        # [C, 6, 48, 48] -> [C, 8hw, 8ww, 216] with token order (d, h, w)
        s = x[0, :, 6 * m:6 * m + 6]
        s = s.reshape(C, 6, 8, 6, 8, 6)
        s = np.transpose(s, (0, 2, 4, 1, 3, 5))
        return np.ascontiguousarray(s.reshape(C, 8, 8, T))

    in_maps = []
    for m in range(NCORES):
        im = {
            "q_slab": to_windows(q_map, m),
            "k_slab": to_windows(k_map, m),
            "v_slab": to_windows(v_map, m),
            "wt0": wt0.astype(np.float16),
            "wt1": wt1.astype(np.float16),
            "pbias": np.ascontiguousarray(bias.reshape(C, 1)),
        }
        if not trivial_q:
            im["gq"] = np.ascontiguousarray(gamma_q.reshape(C, 1))
            im["bq"] = np.ascontiguousarray(beta_q.reshape(C, 1))
        in_maps.append(im)
    return (trivial_q, trivial_bias), in_maps


def _run(inputs, trace=False, **trace_kwargs):
    flags, in_maps = _prepare(inputs)
    nc = _build_nc(*flags)
    res = run_bass_kernel_spmd(nc, in_maps, list(range(NCORES)),
                               trace=trace, **trace_kwargs)
    slabs = []
    for m in range(NCORES):
        s = res.results[m]["y_slab"].reshape(C, 8, 8, 6, 6, 6)
        s = np.transpose(s, (0, 3, 1, 4, 2, 5)).reshape(C, 6, 48, 48)
        slabs.append(s)
    out = np.concatenate(slabs, axis=1).reshape(1, C, 48, 48, 48)
    return out.astype(np.float32), res


def kernel(**inputs):
    out, _ = _run(inputs, trace=False)
    return out


def kernel_traced(**inputs):
    return _run(inputs, trace=True)
